# revision 32
# baseline (speedup 1.0000x reference)
"""Trainium2 Bass kernel for nn_Mlp_cnn_shift (dense CNN MLP with 3x3 patch-shift
and a softmax-gated mix of two branches).

Strategy
--------
Data-parallel over the 16 (B,T) frames: each of the 8 NeuronCores processes 2
frames end-to-end.  All activations are channel-major ([C, tokens]); `x` is
pre-transposed/cast on the host so no on-device transpose is needed.

Patch-shift handling:
 * forward shift (on xh, HID=1024): xh is stored in a zero-padded token layout
   (row pitch 57 = 56 cols + 1 zero pad col, 58-token zero guards per frame)
   and in 9 channel groups of 114 padded to 128 partitions each (host-permuted
   fc_w columns / fc1_w+fc2_w rows).  Every (dh,dw) roll then becomes a pure
   token offset in the fc1/fc2 matmul rhs access pattern, with the zero
   padding reproducing the reference's zero-fill boundary exactly.
 * inverse shift (on gelu(y), C=512): fc1's output y is evacuated in PLAIN
   channel layout (4 blocks of 128) into the same padded token layout; the
   inverse shift is then a single uniform token offset per channel group,
   applied by 12 strided SBUF->SBUF DMA copies (one per group x 128-block
   fragment) into a token-contiguous h, with y's zero pads landing exactly on
   the shift-clipped cells.  This keeps fc1/fc2/proj outputs unpadded (512
   rows, not 576), saving ~25% of their matmul columns vs a padded-576 layout.

The only cross-core coupling is the global (T,H,W) mean feeding the softmax
gate.  Each core only needs its OWN batch element's gate (cores 0-3 hold
batch 0, cores 4-7 batch 1), so the mean is reduced with per-frame AllReduces
over 4-core replica groups; frame 0's collective is absorbed under frame-1
compute, and d = h - w is precomputed during phase B so the post-collective
tail is only scale+add+proj.

bf16 matmuls with f32 PSUM accumulation; output f32.  Each frame's w branch
spills to DRAM (bf16) and streams back during the output phase to fit SBUF.
"""

import os
import sys

for _p in ("/opt/trn_rl_repo",):
    if os.path.isdir(_p) and _p not in sys.path:
        sys.path.append(_p)

import numpy as np
import ml_dtypes

import concourse.bass as bass  # noqa: F401
import concourse.mybir as mybir
import concourse.tile as tile
from concourse import bacc
from concourse.bass_utils import run_bass_kernel_spmd

# ---------------------------------------------------------------- constants
SHIFTS = [(1, 1), (1, 0), (1, -1), (0, 1), (0, 0), (0, -1), (-1, 1), (-1, 0), (-1, -1)]
NG = 9
B, T, H, W, C = 2, 8, 56, 56, 512
HID = 1024
NCORES = 8
NF = (B * T) // NCORES          # frames per core = 2
HWTOK = H * W                   # 3136 tokens per frame
RP = W + 1                      # padded row pitch = 57
GUARD = RP + 1                  # 58 zero tokens on each end
FRPAD = RP * H                  # 3192
XHSPAN = GUARD + FRPAD + GUARD  # 3308
RG = 7                          # row groups per frame
RGR = H // RG                   # 8 rows per group
RGT = RGR * W                   # 448 valid tokens per row group
RGP = RGR * RP                  # 456 padded tokens per row group
GS_HID = 114                    # hid shift-group size (9*114 = 1026 >= 1024)
GS_C = 57                       # C shift-group size (9*57 = 513 >= 512)
CCB = C // 128                  # 4 channel blocks (plain)
MEAN_N = float(T * H * W)
NJUNK = 95                     # PE warm-keeper matmuls over the AllReduce gap

F32 = mybir.dt.float32
BF16 = mybir.dt.bfloat16
BF16_NP = ml_dtypes.bfloat16

_CACHE = {}


def _c_frags():
    """(kb, p0, p1, sh, sw) fragments: C shift groups split at 128-boundaries."""
    out = []
    for g in range(NG):
        c0 = GS_C * g
        c1 = min(GS_C * (g + 1), C)
        sh, sw = SHIFTS[g]
        while c0 < c1:
            kb = c0 // 128
            ce = min(c1, (kb + 1) * 128)
            out.append((kb, c0 - kb * 128, ce - kb * 128, sh, sw))
            c0 = ce
    return out


# ---------------------------------------------------------------- device kernel
def build_nc():
    nc = bacc.Bacc("TRN2", target_bir_lowering=False, debug=False, num_devices=NCORES)

    dp = nc.declare_dram_parameter
    xT = dp("xT", [NF, 128, CCB, HWTOK], BF16, isOutput=False)
    fcw = dp("fcw", [128, CCB, NG * 128], BF16, isOutput=False)
    fcb = dp("fcb", [128, NG], F32, isOutput=False)
    fc1w = dp("fc1w", [128, NG, C], BF16, isOutput=False)
    fc1b = dp("fc1b", [128, CCB], F32, isOutput=False)
    fc2w = dp("fc2w", [128, NG, C], BF16, isOutput=False)
    fc2b = dp("fc2b", [128, CCB], F32, isOutput=False)
    projw = dp("projw", [128, CCB, C], BF16, isOutput=False)
    rw1w = dp("rw1w", [128, CCB, 128], BF16, isOutput=False)
    rw1b = dp("rw1b", [128, 1], F32, isOutput=False)
    rw2w = dp("rw2w", [128, 2 * CCB * 128], BF16, isOutput=False)
    rw2b = dp("rw2b", [128, CCB], F32, isOutput=False)
    out_d = dp("out", [NF, HWTOK, C], F32, isOutput=True)

    # spill space for the w branch of each frame + collective bounce buffers
    wsp = [nc.dram_tensor(f"wsp{f}", [128, CCB, HWTOK], BF16) for f in range(NF)]
    ccin = [nc.dram_tensor(f"ccin{f}", [128, CCB], F32) for f in range(NF)]
    ccout = [nc.dram_tensor(f"ccout{f}", [128, CCB], F32) for f in range(NF)]

    AF = mybir.ActivationFunctionType
    ALU = mybir.AluOpType
    RGROUPS = [[0, 1, 2, 3], [4, 5, 6, 7]]

    with tile.TileContext(nc, num_cores=NCORES) as tc:
        with (
            tc.tile_pool(name="singles", bufs=1) as singles,
            tc.tile_pool(name="xh_pool", bufs=1) as xh_pool,
            tc.tile_pool(name="y_pool", bufs=1) as y_pool,
            tc.tile_pool(name="h_pool", bufs=2) as h_pool,
            tc.tile_pool(name="w_pool", bufs=2) as w_pool,
            tc.tile_pool(name="xt_pool", bufs=2) as xt_pool,
            tc.tile_pool(name="ostage", bufs=2) as ostage,
            tc.tile_pool(name="dstream", bufs=2) as dstream,
            tc.tile_pool(name="gstage", bufs=3) as gstage,
            tc.tile_pool(name="small", bufs=1) as small,
            tc.tile_pool(name="mmpsum", bufs=8, space="PSUM") as mmpsum,
        ):
            # ---- load weights (resident for the whole kernel)
            def load(name, shape, dtype, src):
                t = singles.tile(shape, dtype, name=name)
                nc.sync.dma_start(out=t, in_=src[:])
                return t

            # only what frame-0's fc pass needs is loaded up front; the rest
            # loads while it runs (keeps the kernel head short).  fcw arrives
            # in per-k chunks so the first matmul only waits for chunk 0.
            fcb_s = load("fcb_s", [128, NG], F32, fcb)
            fcw_s = singles.tile([128, CCB, NG * 128], BF16, name="fcw_s")
            for k in range(CCB):
                nc.sync.dma_start(out=fcw_s[:, k], in_=fcw[:, k])
            _rest = {}

            def load_rest():
                _rest["fc1w_s"] = load("fc1w_s", [128, NG, C], BF16, fc1w)
                _rest["fc1b_s"] = load("fc1b_s", [128, CCB], F32, fc1b)
                _rest["fc2w_s"] = load("fc2w_s", [128, NG, C], BF16, fc2w)
                _rest["fc2b_s"] = load("fc2b_s", [128, CCB], F32, fc2b)
                _rest["projw_s"] = load("projw_s", [128, CCB, C], BF16, projw)
                _rest["rw1w_s"] = load("rw1w_s", [128, CCB, 128], BF16, rw1w)
                _rest["rw1b_s"] = load("rw1b_s", [128, 1], F32, rw1b)
                _rest["rw2w_s"] = load("rw2w_s", [128, 2 * CCB * 128], BF16, rw2w)
                _rest["rw2b_s"] = load("rw2b_s", [128, CCB], F32, rw2b)
                # touch Sigmoid once now so its ACT table is resident
                # before the latency-critical gate chain
                warmup = small.tile([128, 1], F32, tag="sgw")
                nc.scalar.activation(
                    out=warmup, in_=_rest["rw1b_s"], func=AF.Sigmoid
                )

            a0_s = singles.tile([128, CCB], F32)   # gate for the h branch

            # xh, padded token layout, persistent across frames.
            xh = xh_pool.tile([128, NG, XHSPAN], BF16)
            # zero guards + per-row pad column once; the body is fully
            # rewritten by every frame's fc pass.
            nc.vector.memset(xh[:, :, :GUARD], 0.0)
            nc.vector.memset(xh[:, :, GUARD + FRPAD:], 0.0)
            xh_rows = xh[:, :, GUARD:GUARD + FRPAD].rearrange(
                "p g (r c) -> p g r c", c=RP
            )
            nc.vector.memset(xh_rows[:, :, :, W:], 0.0)

            # y = gelu(shift(xh) @ fc1_w + b), PLAIN 4x128 channels, padded
            # token layout (guards+pads zeroed once -- they supply the zero
            # fill of the inverse shift; the body is rewritten per frame)
            y = y_pool.tile([128, CCB, XHSPAN], BF16)
            nc.vector.memset(y[:, :, :GUARD], 0.0)
            nc.vector.memset(y[:, :, GUARD + FRPAD:], 0.0)
            y_rows = y[:, :, GUARD:GUARD + FRPAD].rearrange(
                "p g (r c) -> p g r c", c=RP
            )
            nc.vector.memset(y_rows[:, :, :, W:], 0.0)

            hw_tiles = []

            for f in range(NF):
                # ---------------- A: xh = gelu(x @ fc_w + fc_b), group-blocked
                for rg in range(RG):
                    xt_t = xt_pool.tile([128, CCB, RGT], BF16, tag="xt")
                    nc.sync.dma_start(
                        out=xt_t, in_=xT[f, :, :, rg * RGT:(rg + 1) * RGT]
                    )
                    for mb in range(NG):
                        ps = mmpsum.tile([128, 512], F32, tag="mm")
                        for k in range(CCB):
                            nc.tensor.matmul(
                                ps[:, :RGT],
                                lhsT=fcw_s[:, k, mb * 128:(mb + 1) * 128],
                                rhs=xt_t[:, k, :],
                                start=(k == 0),
                                stop=(k == CCB - 1),
                            )
                        dst = xh[
                            :, mb, GUARD + rg * RGP:GUARD + (rg + 1) * RGP
                        ].rearrange("p (r c) -> p r c", c=RP)[:, :, :W]
                        src = ps[:, :RGT].rearrange("p (r c) -> p r c", c=W)
                        nc.scalar.activation(
                            out=dst, in_=src, func=AF.Gelu,
                            bias=fcb_s[:, mb:mb + 1],
                        )

                if f == 0:
                    # frame-0 fc pass is in flight; now bring in the rest
                    load_rest()
                    fc1w_s = _rest["fc1w_s"]; fc1b_s = _rest["fc1b_s"]
                    fc2w_s = _rest["fc2w_s"]; fc2b_s = _rest["fc2b_s"]
                    projw_s = _rest["projw_s"]
                    rw1w_s = _rest["rw1w_s"]; rw1b_s = _rest["rw1b_s"]
                    rw2w_s = _rest["rw2w_s"]; rw2b_s = _rest["rw2b_s"]

                # ---------------- C: y = gelu(shift(xh) @ fc1_w + b), then
                # h = invshift(y) via one contiguous SBUF->SBUF DMA per channel
                # group fragment: in the padded token layout every (sh,sw)
                # roll is a single token offset, and y's zero guards/pads land
                # exactly on the shift-clipped cells.
                h_t = h_pool.tile([128, CCB, FRPAD], BF16, tag="h")
                for rg in range(RG):
                    for mb in range(CCB):
                        ps = mmpsum.tile([128, 512], F32, tag="mm")
                        for g in range(NG):
                            off = -(SHIFTS[g][0] * RP + SHIFTS[g][1])
                            s0 = GUARD + rg * RGP + off
                            rhs2 = xh[:, g, s0:s0 + RGP].rearrange(
                                "p (r c) -> p r c", c=RP
                            )[:, :, :W]
                            nc.tensor.matmul(
                                ps[:, :RGT],
                                lhsT=fc1w_s[:, g, mb * 128:(mb + 1) * 128],
                                rhs=rhs2,
                                start=(g == 0),
                                stop=(g == NG - 1),
                            )
                        dst = y[
                            :, mb, GUARD + rg * RGP:GUARD + (rg + 1) * RGP
                        ].rearrange("p (r c) -> p r c", c=RP)[:, :, :W]
                        src = ps[:, :RGT].rearrange("p (r c) -> p r c", c=W)
                        nc.scalar.activation(
                            out=dst, in_=src, func=AF.Gelu,
                            bias=fc1b_s[:, mb:mb + 1],
                        )
                    # after each row-half of y completes, copy it into h
                    # (two batches per frame keep the copies off B's path)
                    if rg in (3, RG - 1):
                        half0 = 0 if rg == 3 else 28
                        half1 = 28 if rg == 3 else H
                        d0 = half0 * RP
                        d1 = half1 * RP
                        for kb, p0, p1, sh, sw in _c_frags():
                            off = GUARD + sh * RP + sw
                            nc.sync.dma_start(
                                out=h_t[p0:p1, kb, d0:d1],
                                in_=y[p0:p1, kb, d0 + off:d1 + off],
                            )
                        # the copy fills h's pad columns with neighboring row
                        # values; re-zero them (hsum/gating read them)
                        nc.vector.memset(
                            h_t[:, :, d0:d1].rearrange(
                                "p g (r c) -> p g r c", c=RP
                            )[:, :, :, W:],
                            0.0,
                        )

                # ---------------- B: w = gelu(xh @ fc2_w + fc2_b), plain 4x128,
                # spilled to DRAM per row-group; also d = h - w (overwrites h)
                # and the gate partial sums.
                hsum_st = small.tile([128, CCB, RG], F32, tag=f"hsst{f}")
                wsum_st = small.tile([128, CCB, RG], F32, tag=f"wsst{f}")
                nc.vector.memset(wsum_st[:], 0.0)
                for rg in range(RG):
                    w_rg = w_pool.tile([128, CCB, RGT], BF16, tag="wrg")
                    for mb in range(CCB):
                        ps = mmpsum.tile([128, 512], F32, tag="mm")
                        for g in range(NG):
                            s0 = GUARD + rg * RGP
                            rhs2 = xh[:, g, s0:s0 + RGP].rearrange(
                                "p (r c) -> p r c", c=RP
                            )[:, :, :W]
                            nc.tensor.matmul(
                                ps[:, :RGT],
                                lhsT=fc2w_s[:, g, mb * 128:(mb + 1) * 128],
                                rhs=rhs2,
                                start=(g == 0),
                                stop=(g == NG - 1),
                            )
                        nc.scalar.activation(
                            out=w_rg[:, mb, :], in_=ps[:, :RGT], func=AF.Gelu,
                            bias=fc2b_s[:, mb:mb + 1],
                            accum_out=wsum_st[:, mb, rg:rg + 1],
                        )
                    # gate partial sum of h for this row group (pads are 0),
                    # then d = h - w in place on h's valid columns
                    hsl = h_t[:, :, rg * RGP:(rg + 1) * RGP]
                    nc.vector.tensor_reduce(
                        out=hsum_st[:, :, rg:rg + 1].rearrange("p c one -> p (c one)"),
                        in_=hsl, axis=mybir.AxisListType.X, op=ALU.add,
                    )
                    hsl4 = hsl.rearrange("p g (r c) -> p g r c", c=RP)[:, :, :, :W]
                    w4 = w_rg.rearrange("p g (r c) -> p g r c", c=W)
                    nc.vector.tensor_tensor(hsl4, hsl4, w4, ALU.subtract)
                    nc.sync.dma_start(
                        out=wsp[f][:, :, rg * RGT:(rg + 1) * RGT], in_=w_rg[:]
                    )

                # ---------------- per-frame gate partial sum + AllReduce
                # (4-core replica groups: each core only needs its own batch)
                hs = small.tile([128, CCB], F32, tag=f"hs{f}")
                nc.vector.tensor_reduce(
                    out=hs, in_=hsum_st[:], axis=mybir.AxisListType.X, op=ALU.add
                )
                ws = small.tile([128, CCB], F32, tag=f"ws{f}")
                nc.vector.tensor_reduce(
                    out=ws, in_=wsum_st[:], axis=mybir.AxisListType.X, op=ALU.add
                )
                part = small.tile([128, CCB], F32, tag=f"part{f}")
                nc.vector.tensor_tensor(part, hs, ws, ALU.add)
                nc.sync.dma_start(out=ccin[f][:], in_=part)
                nc.gpsimd.collective_compute(
                    "AllReduce",
                    ALU.add,
                    replica_groups=RGROUPS,
                    ins=[ccin[f][:]],
                    outs=[ccout[f][:]],
                )

                hw_tiles.append(h_t)

            # ---------------- combine the two AllReduce results -> z
            # (issued before the warm-keeper so za lands early and zb lands
            # the moment AllReduce-1 completes)
            za = small.tile([128, CCB], F32, tag="za")
            nc.sync.dma_start(out=za, in_=ccout[0][:])
            zb = small.tile([128, CCB], F32, tag="zb")
            nc.sync.dma_start(out=zb, in_=ccout[1][:])
            zsum = small.tile([128, CCB], F32, tag="zsum")
            nc.vector.tensor_tensor(zsum, za, zb, ALU.add)
            zbf = small.tile([128, CCB], BF16, tag="zbf")
            nc.vector.tensor_copy(out=zbf, in_=zsum)

            # keep TensorE's activity monitor warm across the second
            # AllReduce's latency window (junk matmuls, results unread) —
            # otherwise the whole output phase runs at the 4/8 cold clock
            for wi in range(NJUNK):
                wp = mmpsum.tile([128, 512], F32, tag="mm", name=f"warm{wi}")
                nc.tensor.matmul(
                    wp[:, :512],
                    lhsT=fcw_s[:, 0, 0:128],
                    rhs=fcw_s[:, 1, 0:512],
                    start=True,
                    stop=True,
                )

            # ---------------- gate: a = softmax over the 2 streams
            # (1/MEAN_N is folded into rw1w on the host)
            psg = mmpsum.tile([128, 512], F32, tag="mm", name="psg")[:, :1]
            for k in range(CCB):
                nc.tensor.matmul(
                    psg,
                    lhsT=rw1w_s[:, k, :],
                    rhs=zbf[:, k:k + 1],
                    start=(k == 0),
                    stop=(k == CCB - 1),
                )
            gv = small.tile([128, 1], BF16, tag="gv")
            nc.scalar.activation(out=gv, in_=psg, func=AF.Gelu, bias=rw1b_s[:, 0:1])
            psu = mmpsum.tile([128, 512], F32, tag="mm", name="psu")[:, :2 * CCB]
            for m in range(2 * CCB):
                nc.tensor.matmul(
                    psu[:, m:m + 1],
                    lhsT=rw2w_s[:, m * 128:(m + 1) * 128],
                    rhs=gv,
                    start=True,
                    stop=True,
                )
            # softmax over 2 streams == sigmoid of the logit difference:
            # a0 = sigmoid((l0 + b0) - (l1 + b1)); rw2b_s holds b0 - b1
            uv = small.tile([128, 2 * CCB], F32, tag="uv")
            nc.vector.tensor_copy(out=uv, in_=psu)
            ld = small.tile([128, CCB], F32, tag="ld")
            nc.vector.tensor_tensor(ld, uv[:, 0:CCB], uv[:, CCB:2 * CCB], ALU.subtract)
            nc.vector.tensor_tensor(ld, ld, rw2b_s, ALU.add)
            nc.scalar.activation(out=a0_s, in_=ld, func=AF.Sigmoid)

            # ---------------- D: out = (w + a0*d) @ proj_w + proj_b
            # d (= h - w) is already in the h tiles (padded token layout);
            # stream w back per row-group, gate with one ScalarE scale (in
            # place on d) + one VectorE add, then project in 112-token
            # (2 padded rows) M-blocks that skip the pad columns.
            for fidx in range(NF):
                d_t = hw_tiles[fidx]
                # scale d <- a0*d in place, decoupled from the proj pipeline
                # (ScalarE runs ahead; only the add + proj stay in the ring)
                for rg in range(RG):
                    for kb in range(CCB):
                        dck = d_t[:, kb, rg * RGP:(rg + 1) * RGP]
                        nc.scalar.activation(
                            out=dck, in_=dck,
                            func=AF.Copy, scale=a0_s[:, kb:kb + 1],
                        )
                for rg in range(RG):
                    s0 = rg * RGP
                    wc = dstream.tile([128, CCB, RGT], BF16, tag="wc")
                    nc.sync.dma_start(
                        out=wc, in_=wsp[fidx][:, :, rg * RGT:(rg + 1) * RGT]
                    )
                    dck = d_t[:, :, s0:s0 + RGP]
                    # gated = a0*d + w, de-pitched into the token-contiguous
                    # staging tile so the proj lhsT gets plain 2D blocks
                    g_t = gstage.tile([128, CCB, RGT], BF16, tag="gt")
                    nc.vector.tensor_tensor(
                        g_t.rearrange("p g (r c) -> p g r c", c=W),
                        dck.rearrange("p g (r c) -> p g r c", c=RP)[:, :, :, :W],
                        wc.rearrange("p g (r c) -> p g r c", c=W),
                        ALU.add,
                    )
                    for m0 in range(0, RGT, 128):
                        M = min(128, RGT - m0)
                        pp = mmpsum.tile([128, 512], F32, tag="mm")
                        for kb in range(CCB):
                            nc.tensor.matmul(
                                pp[:M, :C],
                                lhsT=g_t[:, kb, m0:m0 + M],
                                rhs=projw_s[:, kb, :],
                                start=(kb == 0),
                                stop=(kb == CCB - 1),
                            )
                        ot = ostage.tile([128, C], F32, tag="ot")
                        if (m0 // 128) % 2 == 0:
                            nc.vector.tensor_copy(out=ot[:M], in_=pp[:M, :C])
                        else:
                            nc.scalar.activation(
                                out=ot[:M], in_=pp[:M, :C], func=AF.Copy
                            )
                        tok = rg * RGT + m0
                        nc.sync.dma_start(
                            out=out_d[fidx, tok:tok + M, :], in_=ot[:M]
                        )

    nc.compile()
    return nc


# ---------------------------------------------------------------- host side
def _prep_weights(fc_w, fc_b, fc1_w, fc1_b, fc2_w, fc2_b,
                  rw1_w, rw1_b, rw2_w, rw2_b, proj_w, proj_b):
    f32 = np.float32

    # fc: columns permuted into 9 HID-groups of 114 (112 for g=8), pad to 128
    fcwp = np.zeros((C, NG * 128), f32)
    fcbp = np.zeros((NG * 128,), f32)
    for g in range(NG):
        n = min(GS_HID * (g + 1), HID) - GS_HID * g
        fcwp[:, 128 * g:128 * g + n] = fc_w[:, GS_HID * g:GS_HID * g + n]
        fcbp[128 * g:128 * g + n] = fc_b[GS_HID * g:GS_HID * g + n]
    fcw_h = np.ascontiguousarray(
        fcwp.reshape(CCB, 128, NG * 128).transpose(1, 0, 2)
    ).astype(BF16_NP)
    fcb_h = np.ascontiguousarray(fcbp.reshape(NG, 128).T).astype(f32)

    def hid_rows_grouped(wm):  # [HID, N] -> [128, NG, N] padded group rows
        wp = np.zeros((NG * 128, wm.shape[1]), f32)
        for g in range(NG):
            n = min(GS_HID * (g + 1), HID) - GS_HID * g
            wp[128 * g:128 * g + n] = wm[GS_HID * g:GS_HID * g + n]
        return np.ascontiguousarray(
            wp.reshape(NG, 128, wm.shape[1]).transpose(1, 0, 2)
        ).astype(BF16_NP)

    fc1w_h = hid_rows_grouped(fc1_w)
    fc2w_h = hid_rows_grouped(fc2_w)

    fc1b_h = np.ascontiguousarray(fc1_b.reshape(CCB, 128).T).astype(f32)
    fc2b_h = np.ascontiguousarray(fc2_b.reshape(CCB, 128).T).astype(f32)

    # proj: plain 4x128 rows, cols plain C
    projw_h = np.ascontiguousarray(
        proj_w.reshape(CCB, 128, C).transpose(1, 0, 2)
    ).astype(BF16_NP)

    # rw1: plain rows, scaled by 1/MEAN_N (folds the mean)
    rw1w_h = np.ascontiguousarray(
        (rw1_w / MEAN_N).reshape(CCB, 128, C // 4).transpose(1, 0, 2)
    ).astype(BF16_NP)
    rw1b_h = np.ascontiguousarray(rw1_b[:, None]).astype(f32)

    # rw2 columns: stream-0 logits (even) in M-blocks 0..3, stream-1 (odd)
    # in M-blocks 4..7
    rw2w_h = np.ascontiguousarray(
        np.concatenate([rw2_w[:, 0::2], rw2_w[:, 1::2]], axis=1)
    ).astype(BF16_NP)
    rw2b_h = np.ascontiguousarray(
        (rw2_b[0::2] - rw2_b[1::2]).reshape(CCB, 128).T
    ).astype(f32)

    return dict(
        fcw=fcw_h, fcb=fcb_h, fc1w=fc1w_h, fc1b=fc1b_h, fc2w=fc2w_h,
        fc2b=fc2b_h, projw=projw_h, rw1w=rw1w_h, rw1b=rw1b_h,
        rw2w=rw2w_h, rw2b=rw2b_h,
    )


def _get_nc():
    if "nc" not in _CACHE:
        _CACHE["nc"] = build_nc()
    return _CACHE["nc"]


def run(inputs, trace=False, trace_kwargs=None):
    """Run the SPMD kernel; returns (full_output, BassKernelResults)."""
    x = np.asarray(inputs["x"], np.float32)
    shared = _prep_weights(
        np.asarray(inputs["fc_w"], np.float32), np.asarray(inputs["fc_b"], np.float32),
        np.asarray(inputs["fc1_w"], np.float32), np.asarray(inputs["fc1_b"], np.float32),
        np.asarray(inputs["fc2_w"], np.float32), np.asarray(inputs["fc2_b"], np.float32),
        np.asarray(inputs["rw1_w"], np.float32), np.asarray(inputs["rw1_b"], np.float32),
        np.asarray(inputs["rw2_w"], np.float32), np.asarray(inputs["rw2_b"], np.float32),
        np.asarray(inputs["proj_w"], np.float32), np.asarray(inputs["proj_b"], np.float32),
    )

    xf = x.reshape(B * T, HWTOK, C)
    in_maps = []
    for c in range(NCORES):
        sh = xf[NF * c:NF * (c + 1)]                      # [NF, 3136, 512]
        xt = sh.transpose(0, 2, 1).reshape(NF, CCB, 128, HWTOK)
        xt = np.ascontiguousarray(xt.transpose(0, 2, 1, 3)).astype(BF16_NP)
        m = dict(shared)
        m["xT"] = xt
        in_maps.append(m)

    nc = _get_nc()
    res = run_bass_kernel_spmd(
        nc, in_maps, list(range(NCORES)),
        trace=trace, **(dict(trace_kwargs=trace_kwargs) if trace_kwargs else {}),
    )

    out = np.empty((B * T, HWTOK, C), np.float32)
    for c in range(NCORES):
        out[NF * c:NF * (c + 1)] = res.results[c]["out"]
    out += np.asarray(inputs["proj_b"], np.float32)  # proj bias, host-side
    return out.reshape(B, T, H, W, C), res


def kernel(**inputs) -> np.ndarray:
    full, _ = run(inputs, trace=False)
    return full


# revision 33
# speedup vs baseline: 1.0568x; 1.0568x over previous
"""Trainium2 Bass kernel for nn_Mlp_cnn_shift (dense CNN MLP with 3x3 patch-shift
and a softmax-gated mix of two branches).

Strategy
--------
Data-parallel over the 16 (B,T) frames: each of the 8 NeuronCores processes 2
frames end-to-end.  All activations are channel-major ([C, tokens]); `x` is
pre-transposed/cast on the host so no on-device transpose is needed.

Patch-shift handling:
 * forward shift (on xh, HID=1024): xh is stored in a zero-padded token layout
   (row pitch 57 = 56 cols + 1 zero pad col, 58-token zero guards per frame)
   and in 9 channel groups of 114 padded to 128 partitions each (host-permuted
   fc_w columns / fc1_w+fc2_w rows).  Every (dh,dw) roll then becomes a pure
   token offset in the fc1/fc2 matmul rhs access pattern, with the zero
   padding reproducing the reference's zero-fill boundary exactly.
 * inverse shift (on gelu(y), C=512): fc1's output y is evacuated in PLAIN
   channel layout (4 blocks of 128) into the same padded token layout; the
   inverse shift is then a single uniform token offset per channel group,
   applied by 12 strided SBUF->SBUF DMA copies (one per group x 128-block
   fragment) into a token-contiguous h, with y's zero pads landing exactly on
   the shift-clipped cells.  This keeps fc1/fc2/proj outputs unpadded (512
   rows, not 576), saving ~25% of their matmul columns vs a padded-576 layout.

The only cross-core coupling is the global (T,H,W) mean feeding the softmax
gate.  Each core only needs its OWN batch element's gate (cores 0-3 hold
batch 0, cores 4-7 batch 1), so the mean is reduced with per-frame AllReduces
over 4-core replica groups; frame 0's collective is absorbed under frame-1
compute, and d = h - w is precomputed during phase B so the post-collective
tail is only scale+add+proj.

bf16 matmuls with f32 PSUM accumulation; output f32.  Each frame's w branch
spills to DRAM (bf16) and streams back during the output phase to fit SBUF.
"""

import os
import sys

for _p in ("/opt/trn_rl_repo",):
    if os.path.isdir(_p) and _p not in sys.path:
        sys.path.append(_p)

import numpy as np
import ml_dtypes

import concourse.bass as bass  # noqa: F401
import concourse.mybir as mybir
import concourse.tile as tile
from concourse import bacc
from concourse.bass_utils import run_bass_kernel_spmd

# ---------------------------------------------------------------- constants
SHIFTS = [(1, 1), (1, 0), (1, -1), (0, 1), (0, 0), (0, -1), (-1, 1), (-1, 0), (-1, -1)]
NG = 9
B, T, H, W, C = 2, 8, 56, 56, 512
HID = 1024
NCORES = 8
NF = (B * T) // NCORES          # frames per core = 2
HWTOK = H * W                   # 3136 tokens per frame
RP = W + 1                      # padded row pitch = 57
GUARD = RP + 1                  # 58 zero tokens on each end
FRPAD = RP * H                  # 3192
XHSPAN = GUARD + FRPAD + GUARD  # 3308
RG = 7                          # row groups per frame
RGR = H // RG                   # 8 rows per group
RGT = RGR * W                   # 448 valid tokens per row group
RGP = RGR * RP                  # 456 padded tokens per row group
GS_HID = 114                    # hid shift-group size (9*114 = 1026 >= 1024)
GS_C = 57                       # C shift-group size (9*57 = 513 >= 512)
CCB = C // 128                  # 4 channel blocks (plain)
MEAN_N = float(T * H * W)
NJUNK = 55                     # PE warm-keeper matmuls over the AllReduce gap

F32 = mybir.dt.float32
BF16 = mybir.dt.bfloat16
BF16_NP = ml_dtypes.bfloat16

_CACHE = {}


def _c_frags():
    """(kb, p0, p1, sh, sw) fragments: C shift groups split at 128-boundaries."""
    out = []
    for g in range(NG):
        c0 = GS_C * g
        c1 = min(GS_C * (g + 1), C)
        sh, sw = SHIFTS[g]
        while c0 < c1:
            kb = c0 // 128
            ce = min(c1, (kb + 1) * 128)
            out.append((kb, c0 - kb * 128, ce - kb * 128, sh, sw))
            c0 = ce
    return out


# ---------------------------------------------------------------- device kernel
def build_nc():
    nc = bacc.Bacc("TRN2", target_bir_lowering=False, debug=False, num_devices=NCORES)

    dp = nc.declare_dram_parameter
    xT = dp("xT", [NF, 128, CCB, HWTOK], BF16, isOutput=False)
    fcw = dp("fcw", [128, CCB, NG * 128], BF16, isOutput=False)
    fcb = dp("fcb", [128, NG], F32, isOutput=False)
    fc1w = dp("fc1w", [128, NG, C], BF16, isOutput=False)
    fc1b = dp("fc1b", [128, CCB], F32, isOutput=False)
    fc2w = dp("fc2w", [128, NG, C], BF16, isOutput=False)
    fc2b = dp("fc2b", [128, CCB], F32, isOutput=False)
    projw = dp("projw", [128, CCB, C], BF16, isOutput=False)
    rw1w = dp("rw1w", [128, CCB, 128], BF16, isOutput=False)
    rw1b = dp("rw1b", [128, 1], F32, isOutput=False)
    rw2w = dp("rw2w", [128, 2 * CCB * 128], BF16, isOutput=False)
    rw2b = dp("rw2b", [128, CCB], F32, isOutput=False)
    out_d = dp("out", [NF, HWTOK, C], F32, isOutput=True)

    # spill space for the w branch of each frame + collective bounce buffers
    wsp = [nc.dram_tensor(f"wsp{f}", [128, CCB, HWTOK], BF16) for f in range(NF)]
    ccin = [nc.dram_tensor(f"ccin{f}", [128, CCB], F32) for f in range(NF)]
    ccout = [nc.dram_tensor(f"ccout{f}", [128, CCB], F32) for f in range(NF)]

    AF = mybir.ActivationFunctionType
    ALU = mybir.AluOpType
    RGROUPS = [[0, 1, 2, 3], [4, 5, 6, 7]]

    with tile.TileContext(nc, num_cores=NCORES) as tc:
        with (
            tc.tile_pool(name="singles", bufs=1) as singles,
            tc.tile_pool(name="xh_pool", bufs=1) as xh_pool,
            tc.tile_pool(name="y_pool", bufs=1) as y_pool,
            tc.tile_pool(name="h_pool", bufs=2) as h_pool,
            tc.tile_pool(name="w_pool", bufs=2) as w_pool,
            tc.tile_pool(name="xt_pool", bufs=2) as xt_pool,
            tc.tile_pool(name="ostage", bufs=3) as ostage,
            tc.tile_pool(name="dstream", bufs=2) as dstream,
            tc.tile_pool(name="gstage", bufs=3) as gstage,
            tc.tile_pool(name="small", bufs=1) as small,
            tc.tile_pool(name="mmpsum", bufs=8, space="PSUM") as mmpsum,
        ):
            # ---- load weights (resident for the whole kernel)
            def load(name, shape, dtype, src):
                t = singles.tile(shape, dtype, name=name)
                nc.sync.dma_start(out=t, in_=src[:])
                return t

            # only what frame-0's fc pass needs is loaded up front; the rest
            # loads while it runs (keeps the kernel head short).  fcw arrives
            # in per-k chunks so the first matmul only waits for chunk 0.
            fcb_s = load("fcb_s", [128, NG], F32, fcb)
            fcw_s = singles.tile([128, CCB, NG * 128], BF16, name="fcw_s")
            for k in range(CCB):
                nc.sync.dma_start(out=fcw_s[:, k], in_=fcw[:, k])
            _rest = {}

            def load_rest():
                _rest["fc1w_s"] = load("fc1w_s", [128, NG, C], BF16, fc1w)
                _rest["fc1b_s"] = load("fc1b_s", [128, CCB], F32, fc1b)
                _rest["fc2w_s"] = load("fc2w_s", [128, NG, C], BF16, fc2w)
                _rest["fc2b_s"] = load("fc2b_s", [128, CCB], F32, fc2b)
                _rest["projw_s"] = load("projw_s", [128, CCB, C], BF16, projw)
                _rest["rw1w_s"] = load("rw1w_s", [128, CCB, 128], BF16, rw1w)
                _rest["rw1b_s"] = load("rw1b_s", [128, 1], F32, rw1b)
                _rest["rw2w_s"] = load("rw2w_s", [128, 2 * CCB * 128], BF16, rw2w)
                _rest["rw2b_s"] = load("rw2b_s", [128, CCB], F32, rw2b)
                # touch Sigmoid once now so its ACT table is resident
                # before the latency-critical gate chain
                warmup = small.tile([128, 1], F32, tag="sgw")
                nc.scalar.activation(
                    out=warmup, in_=_rest["rw1b_s"], func=AF.Sigmoid
                )

            a0_s = singles.tile([128, CCB], F32)   # gate for the h branch

            # xh, padded token layout, persistent across frames.
            xh = xh_pool.tile([128, NG, XHSPAN], BF16)
            # zero guards + per-row pad column once; the body is fully
            # rewritten by every frame's fc pass.
            nc.vector.memset(xh[:, :, :GUARD], 0.0)
            nc.vector.memset(xh[:, :, GUARD + FRPAD:], 0.0)
            xh_rows = xh[:, :, GUARD:GUARD + FRPAD].rearrange(
                "p g (r c) -> p g r c", c=RP
            )
            nc.vector.memset(xh_rows[:, :, :, W:], 0.0)

            # y = gelu(shift(xh) @ fc1_w + b), PLAIN 4x128 channels, padded
            # token layout (guards+pads zeroed once -- they supply the zero
            # fill of the inverse shift; the body is rewritten per frame)
            y = y_pool.tile([128, CCB, XHSPAN], BF16)
            nc.vector.memset(y[:, :, :GUARD], 0.0)
            nc.vector.memset(y[:, :, GUARD + FRPAD:], 0.0)
            y_rows = y[:, :, GUARD:GUARD + FRPAD].rearrange(
                "p g (r c) -> p g r c", c=RP
            )
            nc.vector.memset(y_rows[:, :, :, W:], 0.0)

            hw_tiles = []

            for f in range(NF):
                # ---------------- A: xh = gelu(x @ fc_w + fc_b), group-blocked
                for rg in range(RG):
                    xt_t = xt_pool.tile([128, CCB, RGT], BF16, tag="xt")
                    nc.sync.dma_start(
                        out=xt_t, in_=xT[f, :, :, rg * RGT:(rg + 1) * RGT]
                    )
                    for mb in range(NG):
                        ps = mmpsum.tile([128, 512], F32, tag="mm")
                        for k in range(CCB):
                            nc.tensor.matmul(
                                ps[:, :RGT],
                                lhsT=fcw_s[:, k, mb * 128:(mb + 1) * 128],
                                rhs=xt_t[:, k, :],
                                start=(k == 0),
                                stop=(k == CCB - 1),
                            )
                        dst = xh[
                            :, mb, GUARD + rg * RGP:GUARD + (rg + 1) * RGP
                        ].rearrange("p (r c) -> p r c", c=RP)[:, :, :W]
                        src = ps[:, :RGT].rearrange("p (r c) -> p r c", c=W)
                        nc.scalar.activation(
                            out=dst, in_=src, func=AF.Gelu,
                            bias=fcb_s[:, mb:mb + 1],
                        )

                if f == 0:
                    # frame-0 fc pass is in flight; now bring in the rest
                    load_rest()
                    fc1w_s = _rest["fc1w_s"]; fc1b_s = _rest["fc1b_s"]
                    fc2w_s = _rest["fc2w_s"]; fc2b_s = _rest["fc2b_s"]
                    projw_s = _rest["projw_s"]
                    rw1w_s = _rest["rw1w_s"]; rw1b_s = _rest["rw1b_s"]
                    rw2w_s = _rest["rw2w_s"]; rw2b_s = _rest["rw2b_s"]

                # ---------------- C: y = gelu(shift(xh) @ fc1_w + b), then
                # h = invshift(y) via one contiguous SBUF->SBUF DMA per channel
                # group fragment: in the padded token layout every (sh,sw)
                # roll is a single token offset, and y's zero guards/pads land
                # exactly on the shift-clipped cells.
                h_t = h_pool.tile([128, CCB, FRPAD], BF16, tag="h")
                for rg in range(RG):
                    for mb in range(CCB):
                        ps = mmpsum.tile([128, 512], F32, tag="mm")
                        for g in range(NG):
                            off = -(SHIFTS[g][0] * RP + SHIFTS[g][1])
                            s0 = GUARD + rg * RGP + off
                            rhs2 = xh[:, g, s0:s0 + RGP].rearrange(
                                "p (r c) -> p r c", c=RP
                            )[:, :, :W]
                            nc.tensor.matmul(
                                ps[:, :RGT],
                                lhsT=fc1w_s[:, g, mb * 128:(mb + 1) * 128],
                                rhs=rhs2,
                                start=(g == 0),
                                stop=(g == NG - 1),
                            )
                        dst = y[
                            :, mb, GUARD + rg * RGP:GUARD + (rg + 1) * RGP
                        ].rearrange("p (r c) -> p r c", c=RP)[:, :, :W]
                        src = ps[:, :RGT].rearrange("p (r c) -> p r c", c=W)
                        nc.scalar.activation(
                            out=dst, in_=src, func=AF.Gelu,
                            bias=fc1b_s[:, mb:mb + 1],
                        )
                    # after each row-half of y completes, copy it into h
                    # (two batches per frame keep the copies off B's path)
                    if rg in (3, RG - 1):
                        half0 = 0 if rg == 3 else 28
                        half1 = 28 if rg == 3 else H
                        d0 = half0 * RP
                        d1 = half1 * RP
                        for kb, p0, p1, sh, sw in _c_frags():
                            off = GUARD + sh * RP + sw
                            nc.sync.dma_start(
                                out=h_t[p0:p1, kb, d0:d1],
                                in_=y[p0:p1, kb, d0 + off:d1 + off],
                            )
                        # the copy fills h's pad columns with neighboring row
                        # values; re-zero them (hsum/gating read them)
                        nc.vector.memset(
                            h_t[:, :, d0:d1].rearrange(
                                "p g (r c) -> p g r c", c=RP
                            )[:, :, :, W:],
                            0.0,
                        )

                # ---------------- B: w = gelu(xh @ fc2_w + fc2_b), plain 4x128,
                # spilled to DRAM per row-group; also d = h - w (overwrites h)
                # and the gate partial sums.
                hsum_st = small.tile([128, CCB, RG], F32, tag=f"hsst{f}")
                wsum_st = small.tile([128, CCB, RG], F32, tag=f"wsst{f}")
                nc.vector.memset(wsum_st[:], 0.0)
                for rg in range(RG):
                    w_rg = w_pool.tile([128, CCB, RGT], BF16, tag="wrg")
                    for mb in range(CCB):
                        ps = mmpsum.tile([128, 512], F32, tag="mm")
                        for g in range(NG):
                            s0 = GUARD + rg * RGP
                            rhs2 = xh[:, g, s0:s0 + RGP].rearrange(
                                "p (r c) -> p r c", c=RP
                            )[:, :, :W]
                            nc.tensor.matmul(
                                ps[:, :RGT],
                                lhsT=fc2w_s[:, g, mb * 128:(mb + 1) * 128],
                                rhs=rhs2,
                                start=(g == 0),
                                stop=(g == NG - 1),
                            )
                        nc.scalar.activation(
                            out=w_rg[:, mb, :], in_=ps[:, :RGT], func=AF.Gelu,
                            bias=fc2b_s[:, mb:mb + 1],
                            accum_out=wsum_st[:, mb, rg:rg + 1],
                        )
                    # gate partial sum of h for this row group (pads are 0),
                    # then d = h - w in place on h's valid columns
                    hsl = h_t[:, :, rg * RGP:(rg + 1) * RGP]
                    nc.vector.tensor_reduce(
                        out=hsum_st[:, :, rg:rg + 1].rearrange("p c one -> p (c one)"),
                        in_=hsl, axis=mybir.AxisListType.X, op=ALU.add,
                    )
                    hsl4 = hsl.rearrange("p g (r c) -> p g r c", c=RP)[:, :, :, :W]
                    w4 = w_rg.rearrange("p g (r c) -> p g r c", c=W)
                    nc.vector.tensor_tensor(hsl4, hsl4, w4, ALU.subtract)
                    nc.sync.dma_start(
                        out=wsp[f][:, :, rg * RGT:(rg + 1) * RGT], in_=w_rg[:]
                    )

                # ---------------- per-frame gate partial sum + AllReduce
                # (4-core replica groups: each core only needs its own batch)
                hs = small.tile([128, CCB], F32, tag=f"hs{f}")
                nc.vector.tensor_reduce(
                    out=hs, in_=hsum_st[:], axis=mybir.AxisListType.X, op=ALU.add
                )
                ws = small.tile([128, CCB], F32, tag=f"ws{f}")
                nc.vector.tensor_reduce(
                    out=ws, in_=wsum_st[:], axis=mybir.AxisListType.X, op=ALU.add
                )
                part = small.tile([128, CCB], F32, tag=f"part{f}")
                nc.vector.tensor_tensor(part, hs, ws, ALU.add)
                nc.sync.dma_start(out=ccin[f][:], in_=part)
                nc.gpsimd.collective_compute(
                    "AllReduce",
                    ALU.add,
                    replica_groups=RGROUPS,
                    ins=[ccin[f][:]],
                    outs=[ccout[f][:]],
                )

                hw_tiles.append(h_t)

            # ---------------- combine the two AllReduce results -> z
            # (issued before the warm-keeper so za lands early and zb lands
            # the moment AllReduce-1 completes)
            za = small.tile([128, CCB], F32, tag="za")
            nc.sync.dma_start(out=za, in_=ccout[0][:])
            zb = small.tile([128, CCB], F32, tag="zb")
            nc.sync.dma_start(out=zb, in_=ccout[1][:])
            zsum = small.tile([128, CCB], F32, tag="zsum")
            nc.vector.tensor_tensor(zsum, za, zb, ALU.add)
            zbf = small.tile([128, CCB], BF16, tag="zbf")
            nc.vector.tensor_copy(out=zbf, in_=zsum)

            # keep TensorE's activity monitor warm across the second
            # AllReduce's latency window (junk matmuls, results unread) —
            # otherwise the whole output phase runs at the 4/8 cold clock
            for wi in range(NJUNK):
                wp = mmpsum.tile([128, 512], F32, tag="mm", name=f"warm{wi}")
                nc.tensor.matmul(
                    wp[:, :512],
                    lhsT=fcw_s[:, 0, 0:128],
                    rhs=fcw_s[:, 1, 0:512],
                    start=True,
                    stop=True,
                )

            # ---------------- gate: a = softmax over the 2 streams
            # (1/MEAN_N is folded into rw1w on the host)
            psg = mmpsum.tile([128, 512], F32, tag="mm", name="psg")[:, :1]
            for k in range(CCB):
                nc.tensor.matmul(
                    psg,
                    lhsT=rw1w_s[:, k, :],
                    rhs=zbf[:, k:k + 1],
                    start=(k == 0),
                    stop=(k == CCB - 1),
                )
            gv = small.tile([128, 1], BF16, tag="gv")
            nc.scalar.activation(out=gv, in_=psg, func=AF.Gelu, bias=rw1b_s[:, 0:1])
            psu = mmpsum.tile([128, 512], F32, tag="mm", name="psu")[:, :2 * CCB]
            for m in range(2 * CCB):
                nc.tensor.matmul(
                    psu[:, m:m + 1],
                    lhsT=rw2w_s[:, m * 128:(m + 1) * 128],
                    rhs=gv,
                    start=True,
                    stop=True,
                )
            # softmax over 2 streams == sigmoid of the logit difference:
            # a0 = sigmoid((l0 + b0) - (l1 + b1)); rw2b_s holds b0 - b1
            uv = small.tile([128, 2 * CCB], F32, tag="uv")
            nc.vector.tensor_copy(out=uv, in_=psu)
            ld = small.tile([128, CCB], F32, tag="ld")
            nc.vector.tensor_tensor(ld, uv[:, 0:CCB], uv[:, CCB:2 * CCB], ALU.subtract)
            nc.vector.tensor_tensor(ld, ld, rw2b_s, ALU.add)
            nc.scalar.activation(out=a0_s, in_=ld, func=AF.Sigmoid)

            # ---------------- D: out = (w + a0*d) @ proj_w + proj_b
            # d (= h - w) is already in the h tiles (padded token layout);
            # stream w back per row-group, gate with one ScalarE scale (in
            # place on d) + one VectorE add, then project in 112-token
            # (2 padded rows) M-blocks that skip the pad columns.
            for fidx in range(NF):
                d_t = hw_tiles[fidx]
                # scale d <- a0*d in place, decoupled from the proj pipeline
                # (ScalarE runs ahead; only the add + proj stay in the ring)
                for rg in range(RG):
                    for kb in range(CCB):
                        dck = d_t[:, kb, rg * RGP:(rg + 1) * RGP]
                        nc.scalar.activation(
                            out=dck, in_=dck,
                            func=AF.Copy, scale=a0_s[:, kb:kb + 1],
                        )
                for rg in range(RG):
                    s0 = rg * RGP
                    wc = dstream.tile([128, CCB, RGT], BF16, tag="wc")
                    nc.sync.dma_start(
                        out=wc, in_=wsp[fidx][:, :, rg * RGT:(rg + 1) * RGT]
                    )
                    dck = d_t[:, :, s0:s0 + RGP]
                    # gated = a0*d + w, de-pitched into the token-contiguous
                    # staging tile so the proj lhsT gets plain 2D blocks
                    g_t = gstage.tile([128, CCB, RGT], BF16, tag="gt")
                    nc.vector.tensor_tensor(
                        g_t.rearrange("p g (r c) -> p g r c", c=W),
                        dck.rearrange("p g (r c) -> p g r c", c=RP)[:, :, :, :W],
                        wc.rearrange("p g (r c) -> p g r c", c=W),
                        ALU.add,
                    )
                    for m0 in range(0, RGT, 128):
                        M = min(128, RGT - m0)
                        pp = mmpsum.tile([128, 512], F32, tag="mm")
                        for kb in range(CCB):
                            nc.tensor.matmul(
                                pp[:M, :C],
                                lhsT=g_t[:, kb, m0:m0 + M],
                                rhs=projw_s[:, kb, :],
                                start=(kb == 0),
                                stop=(kb == CCB - 1),
                            )
                        ot = ostage.tile([128, C], F32, tag="ot")
                        nc.vector.tensor_copy(out=ot[:M], in_=pp[:M, :C])
                        tok = rg * RGT + m0
                        nc.sync.dma_start(
                            out=out_d[fidx, tok:tok + M, :], in_=ot[:M]
                        )

    nc.compile()
    return nc


# ---------------------------------------------------------------- host side
def _prep_weights(fc_w, fc_b, fc1_w, fc1_b, fc2_w, fc2_b,
                  rw1_w, rw1_b, rw2_w, rw2_b, proj_w, proj_b):
    f32 = np.float32

    # fc: columns permuted into 9 HID-groups of 114 (112 for g=8), pad to 128
    fcwp = np.zeros((C, NG * 128), f32)
    fcbp = np.zeros((NG * 128,), f32)
    for g in range(NG):
        n = min(GS_HID * (g + 1), HID) - GS_HID * g
        fcwp[:, 128 * g:128 * g + n] = fc_w[:, GS_HID * g:GS_HID * g + n]
        fcbp[128 * g:128 * g + n] = fc_b[GS_HID * g:GS_HID * g + n]
    fcw_h = np.ascontiguousarray(
        fcwp.reshape(CCB, 128, NG * 128).transpose(1, 0, 2)
    ).astype(BF16_NP)
    fcb_h = np.ascontiguousarray(fcbp.reshape(NG, 128).T).astype(f32)

    def hid_rows_grouped(wm):  # [HID, N] -> [128, NG, N] padded group rows
        wp = np.zeros((NG * 128, wm.shape[1]), f32)
        for g in range(NG):
            n = min(GS_HID * (g + 1), HID) - GS_HID * g
            wp[128 * g:128 * g + n] = wm[GS_HID * g:GS_HID * g + n]
        return np.ascontiguousarray(
            wp.reshape(NG, 128, wm.shape[1]).transpose(1, 0, 2)
        ).astype(BF16_NP)

    fc1w_h = hid_rows_grouped(fc1_w)
    fc2w_h = hid_rows_grouped(fc2_w)

    fc1b_h = np.ascontiguousarray(fc1_b.reshape(CCB, 128).T).astype(f32)
    fc2b_h = np.ascontiguousarray(fc2_b.reshape(CCB, 128).T).astype(f32)

    # proj: plain 4x128 rows, cols plain C
    projw_h = np.ascontiguousarray(
        proj_w.reshape(CCB, 128, C).transpose(1, 0, 2)
    ).astype(BF16_NP)

    # rw1: plain rows, scaled by 1/MEAN_N (folds the mean)
    rw1w_h = np.ascontiguousarray(
        (rw1_w / MEAN_N).reshape(CCB, 128, C // 4).transpose(1, 0, 2)
    ).astype(BF16_NP)
    rw1b_h = np.ascontiguousarray(rw1_b[:, None]).astype(f32)

    # rw2 columns: stream-0 logits (even) in M-blocks 0..3, stream-1 (odd)
    # in M-blocks 4..7
    rw2w_h = np.ascontiguousarray(
        np.concatenate([rw2_w[:, 0::2], rw2_w[:, 1::2]], axis=1)
    ).astype(BF16_NP)
    rw2b_h = np.ascontiguousarray(
        (rw2_b[0::2] - rw2_b[1::2]).reshape(CCB, 128).T
    ).astype(f32)

    return dict(
        fcw=fcw_h, fcb=fcb_h, fc1w=fc1w_h, fc1b=fc1b_h, fc2w=fc2w_h,
        fc2b=fc2b_h, projw=projw_h, rw1w=rw1w_h, rw1b=rw1b_h,
        rw2w=rw2w_h, rw2b=rw2b_h,
    )


def _get_nc():
    if "nc" not in _CACHE:
        _CACHE["nc"] = build_nc()
    return _CACHE["nc"]


def run(inputs, trace=False, trace_kwargs=None):
    """Run the SPMD kernel; returns (full_output, BassKernelResults)."""
    x = np.asarray(inputs["x"], np.float32)
    shared = _prep_weights(
        np.asarray(inputs["fc_w"], np.float32), np.asarray(inputs["fc_b"], np.float32),
        np.asarray(inputs["fc1_w"], np.float32), np.asarray(inputs["fc1_b"], np.float32),
        np.asarray(inputs["fc2_w"], np.float32), np.asarray(inputs["fc2_b"], np.float32),
        np.asarray(inputs["rw1_w"], np.float32), np.asarray(inputs["rw1_b"], np.float32),
        np.asarray(inputs["rw2_w"], np.float32), np.asarray(inputs["rw2_b"], np.float32),
        np.asarray(inputs["proj_w"], np.float32), np.asarray(inputs["proj_b"], np.float32),
    )

    xf = x.reshape(B * T, HWTOK, C)
    in_maps = []
    for c in range(NCORES):
        sh = xf[NF * c:NF * (c + 1)]                      # [NF, 3136, 512]
        xt = sh.transpose(0, 2, 1).reshape(NF, CCB, 128, HWTOK)
        xt = np.ascontiguousarray(xt.transpose(0, 2, 1, 3)).astype(BF16_NP)
        m = dict(shared)
        m["xT"] = xt
        in_maps.append(m)

    nc = _get_nc()
    res = run_bass_kernel_spmd(
        nc, in_maps, list(range(NCORES)),
        trace=trace, **(dict(trace_kwargs=trace_kwargs) if trace_kwargs else {}),
    )

    out = np.empty((B * T, HWTOK, C), np.float32)
    for c in range(NCORES):
        out[NF * c:NF * (c + 1)] = res.results[c]["out"]
    out += np.asarray(inputs["proj_b"], np.float32)  # proj bias, host-side
    return out.reshape(B, T, H, W, C), res


def kernel(**inputs) -> np.ndarray:
    full, _ = run(inputs, trace=False)
    return full


# revision 35
# speedup vs baseline: 1.0666x; 1.0092x over previous
"""Trainium2 Bass kernel for nn_Mlp_cnn_shift (dense CNN MLP with 3x3 patch-shift
and a softmax-gated mix of two branches).

Strategy
--------
Data-parallel over the 16 (B,T) frames: each of the 8 NeuronCores processes 2
frames end-to-end.  All activations are channel-major ([C, tokens]); `x` is
pre-transposed/cast on the host so no on-device transpose is needed.

Patch-shift handling:
 * forward shift (on xh, HID=1024): xh is stored in a zero-padded token layout
   (row pitch 57 = 56 cols + 1 zero pad col, 58-token zero guards per frame)
   and in 9 channel groups of 114 padded to 128 partitions each (host-permuted
   fc_w columns / fc1_w+fc2_w rows).  Every (dh,dw) roll then becomes a pure
   token offset in the fc1/fc2 matmul rhs access pattern, with the zero
   padding reproducing the reference's zero-fill boundary exactly.
 * inverse shift (on gelu(y), C=512): fc1's output y is evacuated in PLAIN
   channel layout (4 blocks of 128) into the same padded token layout; the
   inverse shift is then a single uniform token offset per channel group,
   applied by 12 strided SBUF->SBUF DMA copies (one per group x 128-block
   fragment) into a token-contiguous h, with y's zero pads landing exactly on
   the shift-clipped cells.  This keeps fc1/fc2/proj outputs unpadded (512
   rows, not 576), saving ~25% of their matmul columns vs a padded-576 layout.

The only cross-core coupling is the global (T,H,W) mean feeding the softmax
gate.  Each core only needs its OWN batch element's gate (cores 0-3 hold
batch 0, cores 4-7 batch 1), so the mean is reduced with per-frame AllReduces
over 4-core replica groups; frame 0's collective is absorbed under frame-1
compute, and d = h - w is precomputed during phase B so the post-collective
tail is only scale+add+proj.

bf16 matmuls with f32 PSUM accumulation; output f32.  Each frame's w branch
spills to DRAM (bf16) and streams back during the output phase to fit SBUF.
"""

import os
import sys

for _p in ("/opt/trn_rl_repo",):
    if os.path.isdir(_p) and _p not in sys.path:
        sys.path.append(_p)

import numpy as np
import ml_dtypes

import concourse.bass as bass  # noqa: F401
import concourse.mybir as mybir
import concourse.tile as tile
from concourse import bacc
from concourse.bass_utils import run_bass_kernel_spmd

# ---------------------------------------------------------------- constants
SHIFTS = [(1, 1), (1, 0), (1, -1), (0, 1), (0, 0), (0, -1), (-1, 1), (-1, 0), (-1, -1)]
NG = 9
B, T, H, W, C = 2, 8, 56, 56, 512
HID = 1024
NCORES = 8
NF = (B * T) // NCORES          # frames per core = 2
HWTOK = H * W                   # 3136 tokens per frame
RP = W + 1                      # padded row pitch = 57
GUARD = RP + 1                  # 58 zero tokens on each end
FRPAD = RP * H                  # 3192
XHSPAN = GUARD + FRPAD + GUARD  # 3308
RG = 7                          # row groups per frame
RGR = H // RG                   # 8 rows per group
RGT = RGR * W                   # 448 valid tokens per row group
RGP = RGR * RP                  # 456 padded tokens per row group
GS_HID = 114                    # hid shift-group size (9*114 = 1026 >= 1024)
GS_C = 57                       # C shift-group size (9*57 = 513 >= 512)
CCB = C // 128                  # 4 channel blocks (plain)
MEAN_N = float(T * H * W)
NJUNK = 55                     # PE warm-keeper matmuls over the AllReduce gap

F32 = mybir.dt.float32
BF16 = mybir.dt.bfloat16
BF16_NP = ml_dtypes.bfloat16

_CACHE = {}


def _c_frags():
    """(kb, p0, p1, sh, sw) fragments: C shift groups split at 128-boundaries."""
    out = []
    for g in range(NG):
        c0 = GS_C * g
        c1 = min(GS_C * (g + 1), C)
        sh, sw = SHIFTS[g]
        while c0 < c1:
            kb = c0 // 128
            ce = min(c1, (kb + 1) * 128)
            out.append((kb, c0 - kb * 128, ce - kb * 128, sh, sw))
            c0 = ce
    return out


# ---------------------------------------------------------------- device kernel
def build_nc():
    nc = bacc.Bacc("TRN2", target_bir_lowering=False, debug=False, num_devices=NCORES)

    dp = nc.declare_dram_parameter
    xT = dp("xT", [NF, 128, CCB, HWTOK], BF16, isOutput=False)
    fcw = dp("fcw", [128, CCB, NG * 128], BF16, isOutput=False)
    fcb = dp("fcb", [128, NG], F32, isOutput=False)
    fc1w = dp("fc1w", [128, NG, C], BF16, isOutput=False)
    fc1b = dp("fc1b", [128, CCB], F32, isOutput=False)
    fc2w = dp("fc2w", [128, NG, C], BF16, isOutput=False)
    fc2b = dp("fc2b", [128, CCB], F32, isOutput=False)
    projw = dp("projw", [128, CCB, C], BF16, isOutput=False)
    rw1w = dp("rw1w", [128, CCB, 128], BF16, isOutput=False)
    rw1b = dp("rw1b", [128, 1], F32, isOutput=False)
    rw2w = dp("rw2w", [128, 2 * CCB * 128], BF16, isOutput=False)
    rw2b = dp("rw2b", [128, CCB], F32, isOutput=False)
    out_d = dp("out", [NF, HWTOK, C], F32, isOutput=True)

    # spill space for the w branch of each frame + collective bounce buffers
    wsp = [nc.dram_tensor(f"wsp{f}", [128, CCB, HWTOK], BF16) for f in range(NF)]
    ccin = [nc.dram_tensor(f"ccin{f}", [128, CCB], F32) for f in range(NF)]
    ccout = [nc.dram_tensor(f"ccout{f}", [128, CCB], F32) for f in range(NF)]

    AF = mybir.ActivationFunctionType
    ALU = mybir.AluOpType
    RGROUPS = [[0, 1, 2, 3], [4, 5, 6, 7]]

    with tile.TileContext(nc, num_cores=NCORES) as tc:
        with (
            tc.tile_pool(name="singles", bufs=1) as singles,
            tc.tile_pool(name="xh_pool", bufs=1) as xh_pool,
            tc.tile_pool(name="y_pool", bufs=1) as y_pool,
            tc.tile_pool(name="h_pool", bufs=2) as h_pool,
            tc.tile_pool(name="w_pool", bufs=2) as w_pool,
            tc.tile_pool(name="xt_pool", bufs=2) as xt_pool,
            tc.tile_pool(name="ostage", bufs=2) as ostage,
            tc.tile_pool(name="dstream", bufs=2) as dstream,
            tc.tile_pool(name="gstage", bufs=2) as gstage,
            tc.tile_pool(name="small", bufs=1) as small,
            tc.tile_pool(name="mmpsum", bufs=8, space="PSUM") as mmpsum,
        ):
            # ---- load weights (resident for the whole kernel)
            def load(name, shape, dtype, src):
                t = singles.tile(shape, dtype, name=name)
                nc.sync.dma_start(out=t, in_=src[:])
                return t

            # only what frame-0's fc pass needs is loaded up front; the rest
            # loads while it runs (keeps the kernel head short).  fcw arrives
            # in per-k chunks so the first matmul only waits for chunk 0.
            fcb_s = load("fcb_s", [128, NG], F32, fcb)
            fcw_s = singles.tile([128, CCB, NG * 128], BF16, name="fcw_s")
            for k in range(CCB):
                nc.sync.dma_start(out=fcw_s[:, k], in_=fcw[:, k])
            _rest = {}

            def load_rest():
                _rest["fc1w_s"] = load("fc1w_s", [128, NG, C], BF16, fc1w)
                _rest["fc1b_s"] = load("fc1b_s", [128, CCB], F32, fc1b)
                _rest["fc2w_s"] = load("fc2w_s", [128, NG, C], BF16, fc2w)
                _rest["fc2b_s"] = load("fc2b_s", [128, CCB], F32, fc2b)
                _rest["projw_s"] = load("projw_s", [128, CCB, C], BF16, projw)
                _rest["rw1w_s"] = load("rw1w_s", [128, CCB, 128], BF16, rw1w)
                _rest["rw1b_s"] = load("rw1b_s", [128, 1], F32, rw1b)
                _rest["rw2w_s"] = load("rw2w_s", [128, 2 * CCB * 128], BF16, rw2w)
                _rest["rw2b_s"] = load("rw2b_s", [128, CCB], F32, rw2b)
                # touch Sigmoid once now so its ACT table is resident
                # before the latency-critical gate chain
                warmup = small.tile([128, 1], F32, tag="sgw")
                nc.scalar.activation(
                    out=warmup, in_=_rest["rw1b_s"], func=AF.Sigmoid
                )

            a0_s = singles.tile([128, CCB], F32)   # gate for the h branch

            # xh, padded token layout, persistent across frames.
            xh = xh_pool.tile([128, NG, XHSPAN], BF16)
            # zero guards + per-row pad column once; the body is fully
            # rewritten by every frame's fc pass.
            nc.vector.memset(xh[:, :, :GUARD], 0.0)
            nc.vector.memset(xh[:, :, GUARD + FRPAD:], 0.0)
            xh_rows = xh[:, :, GUARD:GUARD + FRPAD].rearrange(
                "p g (r c) -> p g r c", c=RP
            )
            nc.vector.memset(xh_rows[:, :, :, W:], 0.0)

            # y = gelu(shift(xh) @ fc1_w + b), PLAIN 4x128 channels, padded
            # token layout (guards+pads zeroed once -- they supply the zero
            # fill of the inverse shift; the body is rewritten per frame)
            y = y_pool.tile([128, CCB, XHSPAN], BF16)
            nc.vector.memset(y[:, :, :GUARD], 0.0)
            nc.vector.memset(y[:, :, GUARD + FRPAD:], 0.0)
            y_rows = y[:, :, GUARD:GUARD + FRPAD].rearrange(
                "p g (r c) -> p g r c", c=RP
            )
            nc.vector.memset(y_rows[:, :, :, W:], 0.0)

            hw_tiles = []

            for f in range(NF):
                # ---------------- A: xh = gelu(x @ fc_w + fc_b), group-blocked
                for rg in range(RG):
                    xt_t = xt_pool.tile([128, CCB, RGT], BF16, tag="xt")
                    nc.sync.dma_start(
                        out=xt_t, in_=xT[f, :, :, rg * RGT:(rg + 1) * RGT]
                    )
                    for mb in range(NG):
                        ps = mmpsum.tile([128, 512], F32, tag="mm")
                        for k in range(CCB):
                            nc.tensor.matmul(
                                ps[:, :RGT],
                                lhsT=fcw_s[:, k, mb * 128:(mb + 1) * 128],
                                rhs=xt_t[:, k, :],
                                start=(k == 0),
                                stop=(k == CCB - 1),
                            )
                        dst = xh[
                            :, mb, GUARD + rg * RGP:GUARD + (rg + 1) * RGP
                        ].rearrange("p (r c) -> p r c", c=RP)[:, :, :W]
                        src = ps[:, :RGT].rearrange("p (r c) -> p r c", c=W)
                        nc.scalar.activation(
                            out=dst, in_=src, func=AF.Gelu,
                            bias=fcb_s[:, mb:mb + 1],
                        )

                if f == 0:
                    # frame-0 fc pass is in flight; now bring in the rest
                    load_rest()
                    fc1w_s = _rest["fc1w_s"]; fc1b_s = _rest["fc1b_s"]
                    fc2w_s = _rest["fc2w_s"]; fc2b_s = _rest["fc2b_s"]
                    projw_s = _rest["projw_s"]
                    rw1w_s = _rest["rw1w_s"]; rw1b_s = _rest["rw1b_s"]
                    rw2w_s = _rest["rw2w_s"]; rw2b_s = _rest["rw2b_s"]

                # ---------------- C: y = gelu(shift(xh) @ fc1_w + b), then
                # h = invshift(y) via one contiguous SBUF->SBUF DMA per channel
                # group fragment: in the padded token layout every (sh,sw)
                # roll is a single token offset, and y's zero guards/pads land
                # exactly on the shift-clipped cells.
                h_t = h_pool.tile([128, CCB, FRPAD], BF16, tag="h")
                for rg in range(RG):
                    for mb in range(CCB):
                        ps = mmpsum.tile([128, 512], F32, tag="mm")
                        for g in range(NG):
                            off = -(SHIFTS[g][0] * RP + SHIFTS[g][1])
                            s0 = GUARD + rg * RGP + off
                            rhs2 = xh[:, g, s0:s0 + RGP].rearrange(
                                "p (r c) -> p r c", c=RP
                            )[:, :, :W]
                            nc.tensor.matmul(
                                ps[:, :RGT],
                                lhsT=fc1w_s[:, g, mb * 128:(mb + 1) * 128],
                                rhs=rhs2,
                                start=(g == 0),
                                stop=(g == NG - 1),
                            )
                        dst = y[
                            :, mb, GUARD + rg * RGP:GUARD + (rg + 1) * RGP
                        ].rearrange("p (r c) -> p r c", c=RP)[:, :, :W]
                        src = ps[:, :RGT].rearrange("p (r c) -> p r c", c=W)
                        nc.scalar.activation(
                            out=dst, in_=src, func=AF.Gelu,
                            bias=fc1b_s[:, mb:mb + 1],
                        )
                    # after each row-half of y completes, copy it into h
                    # (two batches per frame keep the copies off B's path)
                    if rg in (3, RG - 1):
                        half0 = 0 if rg == 3 else 28
                        half1 = 28 if rg == 3 else H
                        d0 = half0 * RP
                        d1 = half1 * RP
                        for kb, p0, p1, sh, sw in _c_frags():
                            off = GUARD + sh * RP + sw
                            nc.scalar.dma_start(
                                out=h_t[p0:p1, kb, d0:d1],
                                in_=y[p0:p1, kb, d0 + off:d1 + off],
                            )
                        # the copy fills h's pad columns with neighboring row
                        # values; re-zero them (hsum/gating read them)
                        nc.vector.memset(
                            h_t[:, :, d0:d1].rearrange(
                                "p g (r c) -> p g r c", c=RP
                            )[:, :, :, W:],
                            0.0,
                        )

                # ---------------- B: w = gelu(xh @ fc2_w + fc2_b), plain 4x128,
                # spilled to DRAM per row-group; also d = h - w (overwrites h)
                # and the gate partial sums.
                hsum_st = small.tile([128, CCB, RG], F32, tag=f"hsst{f}")
                wsum_st = small.tile([128, CCB, RG], F32, tag=f"wsst{f}")
                nc.vector.memset(wsum_st[:], 0.0)
                for rg in range(RG):
                    w_rg = w_pool.tile([128, CCB, RGT], BF16, tag="wrg")
                    for mb in range(CCB):
                        ps = mmpsum.tile([128, 512], F32, tag="mm")
                        for g in range(NG):
                            s0 = GUARD + rg * RGP
                            rhs2 = xh[:, g, s0:s0 + RGP].rearrange(
                                "p (r c) -> p r c", c=RP
                            )[:, :, :W]
                            nc.tensor.matmul(
                                ps[:, :RGT],
                                lhsT=fc2w_s[:, g, mb * 128:(mb + 1) * 128],
                                rhs=rhs2,
                                start=(g == 0),
                                stop=(g == NG - 1),
                            )
                        nc.scalar.activation(
                            out=w_rg[:, mb, :], in_=ps[:, :RGT], func=AF.Gelu,
                            bias=fc2b_s[:, mb:mb + 1],
                            accum_out=wsum_st[:, mb, rg:rg + 1],
                        )
                    # gate partial sum of h for this row group (pads are 0),
                    # then d = h - w in place on h's valid columns
                    hsl = h_t[:, :, rg * RGP:(rg + 1) * RGP]
                    nc.vector.tensor_reduce(
                        out=hsum_st[:, :, rg:rg + 1].rearrange("p c one -> p (c one)"),
                        in_=hsl, axis=mybir.AxisListType.X, op=ALU.add,
                    )
                    hsl4 = hsl.rearrange("p g (r c) -> p g r c", c=RP)[:, :, :, :W]
                    w4 = w_rg.rearrange("p g (r c) -> p g r c", c=W)
                    nc.vector.tensor_tensor(hsl4, hsl4, w4, ALU.subtract)
                    nc.sync.dma_start(
                        out=wsp[f][:, :, rg * RGT:(rg + 1) * RGT], in_=w_rg[:]
                    )

                # ---------------- per-frame gate partial sum + AllReduce
                # (4-core replica groups: each core only needs its own batch)
                hs = small.tile([128, CCB], F32, tag=f"hs{f}")
                nc.vector.tensor_reduce(
                    out=hs, in_=hsum_st[:], axis=mybir.AxisListType.X, op=ALU.add
                )
                ws = small.tile([128, CCB], F32, tag=f"ws{f}")
                nc.vector.tensor_reduce(
                    out=ws, in_=wsum_st[:], axis=mybir.AxisListType.X, op=ALU.add
                )
                part = small.tile([128, CCB], F32, tag=f"part{f}")
                nc.vector.tensor_tensor(part, hs, ws, ALU.add)
                nc.sync.dma_start(out=ccin[f][:], in_=part)
                nc.gpsimd.collective_compute(
                    "AllReduce",
                    ALU.add,
                    replica_groups=RGROUPS,
                    ins=[ccin[f][:]],
                    outs=[ccout[f][:]],
                )

                hw_tiles.append(h_t)

            # ---------------- combine the two AllReduce results -> z
            # (issued before the warm-keeper so za lands early and zb lands
            # the moment AllReduce-1 completes)
            za = small.tile([128, CCB], F32, tag="za")
            nc.sync.dma_start(out=za, in_=ccout[0][:])
            zb = small.tile([128, CCB], F32, tag="zb")
            nc.sync.dma_start(out=zb, in_=ccout[1][:])
            zsum = small.tile([128, CCB], F32, tag="zsum")
            nc.vector.tensor_tensor(zsum, za, zb, ALU.add)
            zbf = small.tile([128, CCB], BF16, tag="zbf")
            nc.vector.tensor_copy(out=zbf, in_=zsum)

            # keep TensorE's activity monitor warm across the second
            # AllReduce's latency window (junk matmuls, results unread) —
            # otherwise the whole output phase runs at the 4/8 cold clock
            for wi in range(NJUNK):
                wp = mmpsum.tile([128, 512], F32, tag="mm", name=f"warm{wi}")
                nc.tensor.matmul(
                    wp[:, :512],
                    lhsT=fcw_s[:, 0, 0:128],
                    rhs=fcw_s[:, 1, 0:512],
                    start=True,
                    stop=True,
                )

            # ---------------- gate: a = softmax over the 2 streams
            # (1/MEAN_N is folded into rw1w on the host)
            psg = mmpsum.tile([128, 512], F32, tag="mm", name="psg")[:, :1]
            for k in range(CCB):
                nc.tensor.matmul(
                    psg,
                    lhsT=rw1w_s[:, k, :],
                    rhs=zbf[:, k:k + 1],
                    start=(k == 0),
                    stop=(k == CCB - 1),
                )
            gv = small.tile([128, 1], BF16, tag="gv")
            nc.scalar.activation(out=gv, in_=psg, func=AF.Gelu, bias=rw1b_s[:, 0:1])
            psu = mmpsum.tile([128, 512], F32, tag="mm", name="psu")[:, :2 * CCB]
            for m in range(2 * CCB):
                nc.tensor.matmul(
                    psu[:, m:m + 1],
                    lhsT=rw2w_s[:, m * 128:(m + 1) * 128],
                    rhs=gv,
                    start=True,
                    stop=True,
                )
            # softmax over 2 streams == sigmoid of the logit difference:
            # a0 = sigmoid((l0 + b0) - (l1 + b1)); rw2b_s holds b0 - b1
            uv = small.tile([128, 2 * CCB], F32, tag="uv")
            nc.vector.tensor_copy(out=uv, in_=psu)
            ld = small.tile([128, CCB], F32, tag="ld")
            nc.vector.tensor_tensor(ld, uv[:, 0:CCB], uv[:, CCB:2 * CCB], ALU.subtract)
            nc.vector.tensor_tensor(ld, ld, rw2b_s, ALU.add)
            nc.scalar.activation(out=a0_s, in_=ld, func=AF.Sigmoid)

            # ---------------- D: out = (w + a0*d) @ proj_w + proj_b
            # d (= h - w) is already in the h tiles (padded token layout);
            # stream w back per row-group, gate with one ScalarE scale (in
            # place on d) + one VectorE add, then project in 112-token
            # (2 padded rows) M-blocks that skip the pad columns.
            for fidx in range(NF):
                d_t = hw_tiles[fidx]
                # scale d <- a0*d in place, decoupled from the proj pipeline
                # (ScalarE runs ahead; only the add + proj stay in the ring)
                for rg in range(RG):
                    for kb in range(CCB):
                        dck = d_t[:, kb, rg * RGP:(rg + 1) * RGP]
                        nc.scalar.activation(
                            out=dck, in_=dck,
                            func=AF.Copy, scale=a0_s[:, kb:kb + 1],
                        )
                for rg in range(RG):
                    s0 = rg * RGP
                    wc = dstream.tile([128, CCB, RGT], BF16, tag="wc")
                    nc.scalar.dma_start(
                        out=wc, in_=wsp[fidx][:, :, rg * RGT:(rg + 1) * RGT]
                    )
                    dck = d_t[:, :, s0:s0 + RGP]
                    # gated = a0*d + w, de-pitched into the token-contiguous
                    # staging tile so the proj lhsT gets plain 2D blocks
                    g_t = gstage.tile([128, CCB, RGT], BF16, tag="gt")
                    nc.vector.tensor_tensor(
                        g_t.rearrange("p g (r c) -> p g r c", c=W),
                        dck.rearrange("p g (r c) -> p g r c", c=RP)[:, :, :, :W],
                        wc.rearrange("p g (r c) -> p g r c", c=W),
                        ALU.add,
                    )
                    M = RGT // 4  # 112: uniform M-blocks pair cleanly
                    for half in range(2):
                        ot = ostage.tile([128, 2, C], F32, tag="ot")
                        for j in range(2):
                            m0 = (2 * half + j) * M
                            pp = mmpsum.tile([128, 512], F32, tag="mm")
                            for kb in range(CCB):
                                nc.tensor.matmul(
                                    pp[:M, :C],
                                    lhsT=g_t[:, kb, m0:m0 + M],
                                    rhs=projw_s[:, kb, :],
                                    start=(kb == 0),
                                    stop=(kb == CCB - 1),
                                )
                            nc.vector.tensor_copy(
                                out=ot[:M, j, :], in_=pp[:M, :C]
                            )
                        tok = rg * RGT + 2 * half * M
                        dst = out_d[fidx, tok:tok + 2 * M, :].rearrange(
                            "(b t) c -> t b c", b=2
                        )
                        nc.sync.dma_start(out=dst, in_=ot[:M])

    nc.compile()
    return nc


# ---------------------------------------------------------------- host side
def _prep_weights(fc_w, fc_b, fc1_w, fc1_b, fc2_w, fc2_b,
                  rw1_w, rw1_b, rw2_w, rw2_b, proj_w, proj_b):
    f32 = np.float32

    # fc: columns permuted into 9 HID-groups of 114 (112 for g=8), pad to 128
    fcwp = np.zeros((C, NG * 128), f32)
    fcbp = np.zeros((NG * 128,), f32)
    for g in range(NG):
        n = min(GS_HID * (g + 1), HID) - GS_HID * g
        fcwp[:, 128 * g:128 * g + n] = fc_w[:, GS_HID * g:GS_HID * g + n]
        fcbp[128 * g:128 * g + n] = fc_b[GS_HID * g:GS_HID * g + n]
    fcw_h = np.ascontiguousarray(
        fcwp.reshape(CCB, 128, NG * 128).transpose(1, 0, 2)
    ).astype(BF16_NP)
    fcb_h = np.ascontiguousarray(fcbp.reshape(NG, 128).T).astype(f32)

    def hid_rows_grouped(wm):  # [HID, N] -> [128, NG, N] padded group rows
        wp = np.zeros((NG * 128, wm.shape[1]), f32)
        for g in range(NG):
            n = min(GS_HID * (g + 1), HID) - GS_HID * g
            wp[128 * g:128 * g + n] = wm[GS_HID * g:GS_HID * g + n]
        return np.ascontiguousarray(
            wp.reshape(NG, 128, wm.shape[1]).transpose(1, 0, 2)
        ).astype(BF16_NP)

    fc1w_h = hid_rows_grouped(fc1_w)
    fc2w_h = hid_rows_grouped(fc2_w)

    fc1b_h = np.ascontiguousarray(fc1_b.reshape(CCB, 128).T).astype(f32)
    fc2b_h = np.ascontiguousarray(fc2_b.reshape(CCB, 128).T).astype(f32)

    # proj: plain 4x128 rows, cols plain C
    projw_h = np.ascontiguousarray(
        proj_w.reshape(CCB, 128, C).transpose(1, 0, 2)
    ).astype(BF16_NP)

    # rw1: plain rows, scaled by 1/MEAN_N (folds the mean)
    rw1w_h = np.ascontiguousarray(
        (rw1_w / MEAN_N).reshape(CCB, 128, C // 4).transpose(1, 0, 2)
    ).astype(BF16_NP)
    rw1b_h = np.ascontiguousarray(rw1_b[:, None]).astype(f32)

    # rw2 columns: stream-0 logits (even) in M-blocks 0..3, stream-1 (odd)
    # in M-blocks 4..7
    rw2w_h = np.ascontiguousarray(
        np.concatenate([rw2_w[:, 0::2], rw2_w[:, 1::2]], axis=1)
    ).astype(BF16_NP)
    rw2b_h = np.ascontiguousarray(
        (rw2_b[0::2] - rw2_b[1::2]).reshape(CCB, 128).T
    ).astype(f32)

    return dict(
        fcw=fcw_h, fcb=fcb_h, fc1w=fc1w_h, fc1b=fc1b_h, fc2w=fc2w_h,
        fc2b=fc2b_h, projw=projw_h, rw1w=rw1w_h, rw1b=rw1b_h,
        rw2w=rw2w_h, rw2b=rw2b_h,
    )


def _get_nc():
    if "nc" not in _CACHE:
        _CACHE["nc"] = build_nc()
    return _CACHE["nc"]


def run(inputs, trace=False, trace_kwargs=None):
    """Run the SPMD kernel; returns (full_output, BassKernelResults)."""
    x = np.asarray(inputs["x"], np.float32)
    shared = _prep_weights(
        np.asarray(inputs["fc_w"], np.float32), np.asarray(inputs["fc_b"], np.float32),
        np.asarray(inputs["fc1_w"], np.float32), np.asarray(inputs["fc1_b"], np.float32),
        np.asarray(inputs["fc2_w"], np.float32), np.asarray(inputs["fc2_b"], np.float32),
        np.asarray(inputs["rw1_w"], np.float32), np.asarray(inputs["rw1_b"], np.float32),
        np.asarray(inputs["rw2_w"], np.float32), np.asarray(inputs["rw2_b"], np.float32),
        np.asarray(inputs["proj_w"], np.float32), np.asarray(inputs["proj_b"], np.float32),
    )

    xf = x.reshape(B * T, HWTOK, C)
    in_maps = []
    for c in range(NCORES):
        sh = xf[NF * c:NF * (c + 1)]                      # [NF, 3136, 512]
        xt = sh.transpose(0, 2, 1).reshape(NF, CCB, 128, HWTOK)
        xt = np.ascontiguousarray(xt.transpose(0, 2, 1, 3)).astype(BF16_NP)
        m = dict(shared)
        m["xT"] = xt
        in_maps.append(m)

    nc = _get_nc()
    res = run_bass_kernel_spmd(
        nc, in_maps, list(range(NCORES)),
        trace=trace, **(dict(trace_kwargs=trace_kwargs) if trace_kwargs else {}),
    )

    out = np.empty((B * T, HWTOK, C), np.float32)
    for c in range(NCORES):
        out[NF * c:NF * (c + 1)] = res.results[c]["out"]
    out += np.asarray(inputs["proj_b"], np.float32)  # proj bias, host-side
    return out.reshape(B, T, H, W, C), res


def kernel(**inputs) -> np.ndarray:
    full, _ = run(inputs, trace=False)
    return full


# revision 37
# speedup vs baseline: 1.1043x; 1.0353x over previous
"""Trainium2 Bass kernel for nn_Mlp_cnn_shift (dense CNN MLP with 3x3 patch-shift
and a softmax-gated mix of two branches).

Strategy
--------
Data-parallel over the 16 (B,T) frames: each of the 8 NeuronCores processes 2
frames end-to-end.  All activations are channel-major ([C, tokens]); `x` is
pre-transposed/cast on the host so no on-device transpose is needed.

Patch-shift handling:
 * forward shift (on xh, HID=1024): xh is stored in a zero-padded token layout
   (row pitch 57 = 56 cols + 1 zero pad col, 58-token zero guards per frame)
   and in 9 channel groups of 114 padded to 128 partitions each (host-permuted
   fc_w columns / fc1_w+fc2_w rows).  Every (dh,dw) roll then becomes a pure
   token offset in the fc1/fc2 matmul rhs access pattern, with the zero
   padding reproducing the reference's zero-fill boundary exactly.
 * inverse shift (on gelu(y), C=512): fc1's output y is evacuated in PLAIN
   channel layout (4 blocks of 128) into the same padded token layout; the
   inverse shift is then a single uniform token offset per channel group,
   applied by 12 strided SBUF->SBUF DMA copies (one per group x 128-block
   fragment) into a token-contiguous h, with y's zero pads landing exactly on
   the shift-clipped cells.  This keeps fc1/fc2/proj outputs unpadded (512
   rows, not 576), saving ~25% of their matmul columns vs a padded-576 layout.

The only cross-core coupling is the global (T,H,W) mean feeding the softmax
gate.  Each core only needs its OWN batch element's gate (cores 0-3 hold
batch 0, cores 4-7 batch 1), so the mean is reduced with per-frame AllReduces
over 4-core replica groups; frame 0's collective is absorbed under frame-1
compute, and d = h - w is precomputed during phase B so the post-collective
tail is only scale+add+proj.

bf16 matmuls with f32 PSUM accumulation; output f32.  Each frame's w branch
spills to DRAM (bf16) and streams back during the output phase to fit SBUF.
"""

import os
import sys

for _p in ("/opt/trn_rl_repo",):
    if os.path.isdir(_p) and _p not in sys.path:
        sys.path.append(_p)

import numpy as np
import ml_dtypes

import concourse.bass as bass  # noqa: F401
import concourse.mybir as mybir
import concourse.tile as tile
from concourse import bacc
from concourse.bass_utils import run_bass_kernel_spmd

# ---------------------------------------------------------------- constants
SHIFTS = [(1, 1), (1, 0), (1, -1), (0, 1), (0, 0), (0, -1), (-1, 1), (-1, 0), (-1, -1)]
NG = 9
B, T, H, W, C = 2, 8, 56, 56, 512
HID = 1024
NCORES = 8
NF = (B * T) // NCORES          # frames per core = 2
HWTOK = H * W                   # 3136 tokens per frame
RP = W + 1                      # padded row pitch = 57
GUARD = RP + 1                  # 58 zero tokens on each end
FRPAD = RP * H                  # 3192
XHSPAN = GUARD + FRPAD + GUARD  # 3308
RG = 7                          # row groups per frame
RGR = H // RG                   # 8 rows per group
RGT = RGR * W                   # 448 valid tokens per row group
RGP = RGR * RP                  # 456 padded tokens per row group
GS_HID = 114                    # hid shift-group size (9*114 = 1026 >= 1024)
GS_C = 57                       # C shift-group size (9*57 = 513 >= 512)
CCB = C // 128                  # 4 channel blocks (plain)
MEAN_N = float(T * H * W)
NJUNK = 80                     # PE warm-keeper matmuls over the AllReduce gap

F32 = mybir.dt.float32
BF16 = mybir.dt.bfloat16
BF16_NP = ml_dtypes.bfloat16

_CACHE = {}


def _c_frags():
    """(kb, p0, p1, sh, sw) fragments: C shift groups split at 128-boundaries."""
    out = []
    for g in range(NG):
        c0 = GS_C * g
        c1 = min(GS_C * (g + 1), C)
        sh, sw = SHIFTS[g]
        while c0 < c1:
            kb = c0 // 128
            ce = min(c1, (kb + 1) * 128)
            out.append((kb, c0 - kb * 128, ce - kb * 128, sh, sw))
            c0 = ce
    return out


# ---------------------------------------------------------------- device kernel
def build_nc():
    nc = bacc.Bacc("TRN2", target_bir_lowering=False, debug=False, num_devices=NCORES)

    dp = nc.declare_dram_parameter
    xT = dp("xT", [NF, 128, CCB, HWTOK], BF16, isOutput=False)
    fcw = dp("fcw", [128, CCB, NG * 128], BF16, isOutput=False)
    fcb = dp("fcb", [128, NG], F32, isOutput=False)
    fc1w = dp("fc1w", [128, NG, C], BF16, isOutput=False)
    fc1b = dp("fc1b", [128, CCB], F32, isOutput=False)
    fc2w = dp("fc2w", [128, NG, C], BF16, isOutput=False)
    fc2b = dp("fc2b", [128, CCB], F32, isOutput=False)
    projw = dp("projw", [128, CCB, C], BF16, isOutput=False)
    rw1w = dp("rw1w", [128, CCB, 128], BF16, isOutput=False)
    rw1b = dp("rw1b", [128, 1], F32, isOutput=False)
    rw2w = dp("rw2w", [128, 2 * CCB * 128], BF16, isOutput=False)
    rw2b = dp("rw2b", [128, CCB], F32, isOutput=False)
    out_d = dp("out", [NF, HWTOK, C], F32, isOutput=True)

    # spill space for the w branch of each frame + collective bounce buffers
    wsp = [nc.dram_tensor(f"wsp{f}", [128, CCB, HWTOK], BF16) for f in range(NF)]
    ccin = [nc.dram_tensor(f"ccin{f}", [128, CCB], F32) for f in range(NF)]
    ccout = [nc.dram_tensor(f"ccout{f}", [128, CCB], F32) for f in range(NF)]

    AF = mybir.ActivationFunctionType
    ALU = mybir.AluOpType
    RGROUPS = [[0, 1, 2, 3], [4, 5, 6, 7]]

    with tile.TileContext(nc, num_cores=NCORES) as tc:
        with (
            tc.tile_pool(name="singles", bufs=1) as singles,
            tc.tile_pool(name="xh_pool", bufs=1) as xh_pool,
            tc.tile_pool(name="y_pool", bufs=1) as y_pool,
            tc.tile_pool(name="h_pool", bufs=2) as h_pool,
            tc.tile_pool(name="w_pool", bufs=2) as w_pool,
            tc.tile_pool(name="xt_pool", bufs=2) as xt_pool,
            tc.tile_pool(name="ostage", bufs=2) as ostage,
            tc.tile_pool(name="dstream", bufs=2) as dstream,
            tc.tile_pool(name="gstage", bufs=2) as gstage,
            tc.tile_pool(name="small", bufs=1) as small,
            tc.tile_pool(name="mmpsum", bufs=8, space="PSUM") as mmpsum,
        ):
            # ---- load weights (resident for the whole kernel)
            def load(name, shape, dtype, src):
                t = singles.tile(shape, dtype, name=name)
                nc.sync.dma_start(out=t, in_=src[:])
                return t

            # only what frame-0's fc pass needs is loaded up front; the rest
            # loads while it runs (keeps the kernel head short).  fcw arrives
            # in per-k chunks so the first matmul only waits for chunk 0.
            fcb_s = load("fcb_s", [128, NG], F32, fcb)
            fcw_s = singles.tile([128, CCB, NG * 128], BF16, name="fcw_s")
            for k in range(CCB):
                nc.sync.dma_start(out=fcw_s[:, k], in_=fcw[:, k])
            _rest = {}

            def load_rest():
                _rest["fc1w_s"] = load("fc1w_s", [128, NG, C], BF16, fc1w)
                _rest["fc1b_s"] = load("fc1b_s", [128, CCB], F32, fc1b)
                _rest["fc2w_s"] = load("fc2w_s", [128, NG, C], BF16, fc2w)
                _rest["fc2b_s"] = load("fc2b_s", [128, CCB], F32, fc2b)
                _rest["projw_s"] = load("projw_s", [128, CCB, C], BF16, projw)
                _rest["rw1w_s"] = load("rw1w_s", [128, CCB, 128], BF16, rw1w)
                _rest["rw1b_s"] = load("rw1b_s", [128, 1], F32, rw1b)
                _rest["rw2w_s"] = load("rw2w_s", [128, 2 * CCB * 128], BF16, rw2w)
                _rest["rw2b_s"] = load("rw2b_s", [128, CCB], F32, rw2b)
                # touch Sigmoid once now so its ACT table is resident
                # before the latency-critical gate chain
                warmup = small.tile([128, 1], F32, tag="sgw")
                nc.scalar.activation(
                    out=warmup, in_=_rest["rw1b_s"], func=AF.Sigmoid
                )

            a0_s = singles.tile([128, CCB], F32)   # gate for the h branch

            # xh, padded token layout, persistent across frames.
            xh = xh_pool.tile([128, NG, XHSPAN], BF16)
            # zero guards + per-row pad column once; the body is fully
            # rewritten by every frame's fc pass.
            nc.vector.memset(xh[:, :, :GUARD], 0.0)
            nc.vector.memset(xh[:, :, GUARD + FRPAD:], 0.0)
            xh_rows = xh[:, :, GUARD:GUARD + FRPAD].rearrange(
                "p g (r c) -> p g r c", c=RP
            )
            nc.vector.memset(xh_rows[:, :, :, W:], 0.0)

            # y = gelu(shift(xh) @ fc1_w + b), PLAIN 4x128 channels, padded
            # token layout (guards+pads zeroed once -- they supply the zero
            # fill of the inverse shift; the body is rewritten per frame)
            y = y_pool.tile([128, CCB, XHSPAN], BF16)
            nc.vector.memset(y[:, :, :GUARD], 0.0)
            nc.vector.memset(y[:, :, GUARD + FRPAD:], 0.0)
            y_rows = y[:, :, GUARD:GUARD + FRPAD].rearrange(
                "p g (r c) -> p g r c", c=RP
            )
            nc.vector.memset(y_rows[:, :, :, W:], 0.0)

            hw_tiles = []

            for f in range(NF):
                # ---------------- A: xh = gelu(x @ fc_w + fc_b), group-blocked
                for rg in range(RG):
                    xt_t = xt_pool.tile([128, CCB, RGT], BF16, tag="xt")
                    nc.sync.dma_start(
                        out=xt_t, in_=xT[f, :, :, rg * RGT:(rg + 1) * RGT]
                    )
                    for mb in range(NG):
                        ps = mmpsum.tile([128, 512], F32, tag="mm")
                        for k in range(CCB):
                            nc.tensor.matmul(
                                ps[:, :RGT],
                                lhsT=fcw_s[:, k, mb * 128:(mb + 1) * 128],
                                rhs=xt_t[:, k, :],
                                start=(k == 0),
                                stop=(k == CCB - 1),
                            )
                        dst = xh[
                            :, mb, GUARD + rg * RGP:GUARD + (rg + 1) * RGP
                        ].rearrange("p (r c) -> p r c", c=RP)[:, :, :W]
                        src = ps[:, :RGT].rearrange("p (r c) -> p r c", c=W)
                        nc.scalar.activation(
                            out=dst, in_=src, func=AF.Gelu,
                            bias=fcb_s[:, mb:mb + 1],
                        )

                if f == 0:
                    # frame-0 fc pass is in flight; now bring in the rest
                    load_rest()
                    fc1w_s = _rest["fc1w_s"]; fc1b_s = _rest["fc1b_s"]
                    fc2w_s = _rest["fc2w_s"]; fc2b_s = _rest["fc2b_s"]
                    projw_s = _rest["projw_s"]
                    rw1w_s = _rest["rw1w_s"]; rw1b_s = _rest["rw1b_s"]
                    rw2w_s = _rest["rw2w_s"]; rw2b_s = _rest["rw2b_s"]

                # ---------------- C: y = gelu(shift(xh) @ fc1_w + b), then
                # h = invshift(y) via one contiguous SBUF->SBUF DMA per channel
                # group fragment: in the padded token layout every (sh,sw)
                # roll is a single token offset, and y's zero guards/pads land
                # exactly on the shift-clipped cells.
                h_t = h_pool.tile([128, CCB, FRPAD], BF16, tag="h")
                for rg in range(RG):
                    for mb in range(CCB):
                        ps = mmpsum.tile([128, 512], F32, tag="mm")
                        for g in range(NG):
                            off = -(SHIFTS[g][0] * RP + SHIFTS[g][1])
                            s0 = GUARD + rg * RGP + off
                            rhs2 = xh[:, g, s0:s0 + RGP].rearrange(
                                "p (r c) -> p r c", c=RP
                            )[:, :, :W]
                            nc.tensor.matmul(
                                ps[:, :RGT],
                                lhsT=fc1w_s[:, g, mb * 128:(mb + 1) * 128],
                                rhs=rhs2,
                                start=(g == 0),
                                stop=(g == NG - 1),
                            )
                        dst = y[
                            :, mb, GUARD + rg * RGP:GUARD + (rg + 1) * RGP
                        ].rearrange("p (r c) -> p r c", c=RP)[:, :, :W]
                        src = ps[:, :RGT].rearrange("p (r c) -> p r c", c=W)
                        nc.scalar.activation(
                            out=dst, in_=src, func=AF.Gelu,
                            bias=fc1b_s[:, mb:mb + 1],
                        )
                    # after each row-half of y completes, copy it into h
                    # (two batches per frame keep the copies off B's path)
                    if rg in (3, RG - 1):
                        half0 = 0 if rg == 3 else 28
                        half1 = 28 if rg == 3 else H
                        d0 = half0 * RP
                        d1 = half1 * RP
                        for kb, p0, p1, sh, sw in _c_frags():
                            off = GUARD + sh * RP + sw
                            nc.scalar.dma_start(
                                out=h_t[p0:p1, kb, d0:d1],
                                in_=y[p0:p1, kb, d0 + off:d1 + off],
                            )
                        # the copy fills h's pad columns with neighboring row
                        # values; re-zero them (hsum/gating read them)
                        nc.vector.memset(
                            h_t[:, :, d0:d1].rearrange(
                                "p g (r c) -> p g r c", c=RP
                            )[:, :, :, W:],
                            0.0,
                        )

                # ---------------- B: w = gelu(xh @ fc2_w + fc2_b), plain 4x128,
                # spilled to DRAM per row-group; also d = h - w (overwrites h)
                # and the gate partial sums.
                hsum_st = small.tile([128, CCB, RG], F32, tag=f"hsst{f}")
                wsum_st = small.tile([128, CCB, RG], F32, tag=f"wsst{f}")
                nc.vector.memset(wsum_st[:], 0.0)
                for rg in range(RG):
                    w_rg = w_pool.tile([128, CCB, RGT], BF16, tag="wrg")
                    for mb in range(CCB):
                        ps = mmpsum.tile([128, 512], F32, tag="mm")
                        for g in range(NG):
                            s0 = GUARD + rg * RGP
                            rhs2 = xh[:, g, s0:s0 + RGP].rearrange(
                                "p (r c) -> p r c", c=RP
                            )[:, :, :W]
                            nc.tensor.matmul(
                                ps[:, :RGT],
                                lhsT=fc2w_s[:, g, mb * 128:(mb + 1) * 128],
                                rhs=rhs2,
                                start=(g == 0),
                                stop=(g == NG - 1),
                            )
                        nc.scalar.activation(
                            out=w_rg[:, mb, :], in_=ps[:, :RGT], func=AF.Gelu,
                            bias=fc2b_s[:, mb:mb + 1],
                            accum_out=wsum_st[:, mb, rg:rg + 1],
                        )
                    # gate partial sum of h for this row group (pads are 0),
                    # then d = h - w in place on h's valid columns
                    hsl = h_t[:, :, rg * RGP:(rg + 1) * RGP]
                    nc.vector.tensor_reduce(
                        out=hsum_st[:, :, rg:rg + 1].rearrange("p c one -> p (c one)"),
                        in_=hsl, axis=mybir.AxisListType.X, op=ALU.add,
                    )
                    hsl4 = hsl.rearrange("p g (r c) -> p g r c", c=RP)[:, :, :, :W]
                    w4 = w_rg.rearrange("p g (r c) -> p g r c", c=W)
                    nc.vector.tensor_tensor(hsl4, hsl4, w4, ALU.subtract)
                    nc.sync.dma_start(
                        out=wsp[f][:, :, rg * RGT:(rg + 1) * RGT], in_=w_rg[:]
                    )

                # ---------------- per-frame gate partial sum + AllReduce
                # (4-core replica groups: each core only needs its own batch)
                hs = small.tile([128, CCB], F32, tag=f"hs{f}")
                nc.vector.tensor_reduce(
                    out=hs, in_=hsum_st[:], axis=mybir.AxisListType.X, op=ALU.add
                )
                ws = small.tile([128, CCB], F32, tag=f"ws{f}")
                nc.vector.tensor_reduce(
                    out=ws, in_=wsum_st[:], axis=mybir.AxisListType.X, op=ALU.add
                )
                part = small.tile([128, CCB], F32, tag=f"part{f}")
                nc.vector.tensor_tensor(part, hs, ws, ALU.add)
                nc.sync.dma_start(out=ccin[f][:], in_=part)
                nc.gpsimd.collective_compute(
                    "AllReduce",
                    ALU.add,
                    replica_groups=RGROUPS,
                    ins=[ccin[f][:]],
                    outs=[ccout[f][:]],
                )

                hw_tiles.append(h_t)

            # ---------------- combine the two AllReduce results -> z
            # (issued before the warm-keeper so za lands early and zb lands
            # the moment AllReduce-1 completes)
            za = small.tile([128, CCB], F32, tag="za")
            nc.sync.dma_start(out=za, in_=ccout[0][:])
            zb = small.tile([128, CCB], F32, tag="zb")
            nc.sync.dma_start(out=zb, in_=ccout[1][:])
            zsum = small.tile([128, CCB], F32, tag="zsum")
            nc.vector.tensor_tensor(zsum, za, zb, ALU.add)
            zbf = small.tile([128, CCB], BF16, tag="zbf")
            nc.vector.tensor_copy(out=zbf, in_=zsum)

            # keep TensorE's activity monitor warm across the second
            # AllReduce's latency window (junk matmuls, results unread) —
            # otherwise the whole output phase runs at the 4/8 cold clock
            for wi in range(NJUNK):
                wp = mmpsum.tile([128, 512], F32, tag="mm", name=f"warm{wi}")
                nc.tensor.matmul(
                    wp[:, :512],
                    lhsT=fcw_s[:, 0, 0:128],
                    rhs=fcw_s[:, 1, 0:512],
                    start=True,
                    stop=True,
                )

            # ---------------- gate: a = softmax over the 2 streams
            # (1/MEAN_N is folded into rw1w on the host)
            psg = mmpsum.tile([128, 512], F32, tag="mm", name="psg")[:, :1]
            for k in range(CCB):
                nc.tensor.matmul(
                    psg,
                    lhsT=rw1w_s[:, k, :],
                    rhs=zbf[:, k:k + 1],
                    start=(k == 0),
                    stop=(k == CCB - 1),
                )
            gv = small.tile([128, 1], BF16, tag="gv")
            nc.scalar.activation(out=gv, in_=psg, func=AF.Gelu, bias=rw1b_s[:, 0:1])
            psu = mmpsum.tile([128, 512], F32, tag="mm", name="psu")[:, :2 * CCB]
            for m in range(2 * CCB):
                nc.tensor.matmul(
                    psu[:, m:m + 1],
                    lhsT=rw2w_s[:, m * 128:(m + 1) * 128],
                    rhs=gv,
                    start=True,
                    stop=True,
                )
            # softmax over 2 streams == sigmoid of the logit difference:
            # a0 = sigmoid((l0 + b0) - (l1 + b1)); rw2b_s holds b0 - b1
            uv = small.tile([128, 2 * CCB], F32, tag="uv")
            nc.vector.tensor_copy(out=uv, in_=psu)
            ld = small.tile([128, CCB], F32, tag="ld")
            nc.vector.tensor_tensor(ld, uv[:, 0:CCB], uv[:, CCB:2 * CCB], ALU.subtract)
            nc.vector.tensor_tensor(ld, ld, rw2b_s, ALU.add)
            nc.scalar.activation(out=a0_s, in_=ld, func=AF.Sigmoid)

            # ---------------- D: out = (w + a0*d) @ proj_w + proj_b
            # d (= h - w) is already in the h tiles (padded token layout);
            # stream w back per row-group, gate with one ScalarE scale (in
            # place on d) + one VectorE add, then project in 112-token
            # (2 padded rows) M-blocks that skip the pad columns.
            for fidx in range(NF):
                d_t = hw_tiles[fidx]
                for rg in range(RG):
                    s0 = rg * RGP
                    wc = dstream.tile([128, CCB, RGT], BF16, tag="wc")
                    nc.sync.dma_start(
                        out=wc, in_=wsp[fidx][:, :, rg * RGT:(rg + 1) * RGT]
                    )
                    dck = d_t[:, :, s0:s0 + RGP]
                    # gated = a0*d + w in one fused VectorE op per k-block,
                    # de-pitched into the token-contiguous staging tile so
                    # the proj lhsT gets plain 2D blocks
                    g_t = gstage.tile([128, CCB, RGT], BF16, tag="gt")
                    for kb in range(CCB):
                        nc.vector.scalar_tensor_tensor(
                            out=g_t[:, kb],
                            in0=dck[:, kb].rearrange(
                                "p (r c) -> p r c", c=RP
                            )[:, :, :W],
                            scalar=a0_s[:, kb:kb + 1],
                            in1=wc[:, kb],
                            op0=ALU.mult,
                            op1=ALU.add,
                        )
                    M = RGT // 4  # 112: uniform M-blocks pair cleanly
                    for half in range(2):
                        ot = ostage.tile([128, 2, C], F32, tag="ot")
                        for j in range(2):
                            m0 = (2 * half + j) * M
                            pp = mmpsum.tile([128, 512], F32, tag="mm")
                            for kb in range(CCB):
                                nc.tensor.matmul(
                                    pp[:M, :C],
                                    lhsT=g_t[:, kb, m0:m0 + M],
                                    rhs=projw_s[:, kb, :],
                                    start=(kb == 0),
                                    stop=(kb == CCB - 1),
                                )
                            if j == 0:
                                nc.vector.tensor_copy(
                                    out=ot[:M, j, :], in_=pp[:M, :C]
                                )
                            else:
                                nc.scalar.activation(
                                    out=ot[:M, j, :], in_=pp[:M, :C],
                                    func=AF.Copy,
                                )
                        tok = rg * RGT + 2 * half * M
                        dst = out_d[fidx, tok:tok + 2 * M, :].rearrange(
                            "(b t) c -> t b c", b=2
                        )
                        nc.sync.dma_start(out=dst, in_=ot[:M])

    nc.compile()
    return nc


# ---------------------------------------------------------------- host side
def _prep_weights(fc_w, fc_b, fc1_w, fc1_b, fc2_w, fc2_b,
                  rw1_w, rw1_b, rw2_w, rw2_b, proj_w, proj_b):
    f32 = np.float32

    # fc: columns permuted into 9 HID-groups of 114 (112 for g=8), pad to 128
    fcwp = np.zeros((C, NG * 128), f32)
    fcbp = np.zeros((NG * 128,), f32)
    for g in range(NG):
        n = min(GS_HID * (g + 1), HID) - GS_HID * g
        fcwp[:, 128 * g:128 * g + n] = fc_w[:, GS_HID * g:GS_HID * g + n]
        fcbp[128 * g:128 * g + n] = fc_b[GS_HID * g:GS_HID * g + n]
    fcw_h = np.ascontiguousarray(
        fcwp.reshape(CCB, 128, NG * 128).transpose(1, 0, 2)
    ).astype(BF16_NP)
    fcb_h = np.ascontiguousarray(fcbp.reshape(NG, 128).T).astype(f32)

    def hid_rows_grouped(wm):  # [HID, N] -> [128, NG, N] padded group rows
        wp = np.zeros((NG * 128, wm.shape[1]), f32)
        for g in range(NG):
            n = min(GS_HID * (g + 1), HID) - GS_HID * g
            wp[128 * g:128 * g + n] = wm[GS_HID * g:GS_HID * g + n]
        return np.ascontiguousarray(
            wp.reshape(NG, 128, wm.shape[1]).transpose(1, 0, 2)
        ).astype(BF16_NP)

    fc1w_h = hid_rows_grouped(fc1_w)
    fc2w_h = hid_rows_grouped(fc2_w)

    fc1b_h = np.ascontiguousarray(fc1_b.reshape(CCB, 128).T).astype(f32)
    fc2b_h = np.ascontiguousarray(fc2_b.reshape(CCB, 128).T).astype(f32)

    # proj: plain 4x128 rows, cols plain C
    projw_h = np.ascontiguousarray(
        proj_w.reshape(CCB, 128, C).transpose(1, 0, 2)
    ).astype(BF16_NP)

    # rw1: plain rows, scaled by 1/MEAN_N (folds the mean)
    rw1w_h = np.ascontiguousarray(
        (rw1_w / MEAN_N).reshape(CCB, 128, C // 4).transpose(1, 0, 2)
    ).astype(BF16_NP)
    rw1b_h = np.ascontiguousarray(rw1_b[:, None]).astype(f32)

    # rw2 columns: stream-0 logits (even) in M-blocks 0..3, stream-1 (odd)
    # in M-blocks 4..7
    rw2w_h = np.ascontiguousarray(
        np.concatenate([rw2_w[:, 0::2], rw2_w[:, 1::2]], axis=1)
    ).astype(BF16_NP)
    rw2b_h = np.ascontiguousarray(
        (rw2_b[0::2] - rw2_b[1::2]).reshape(CCB, 128).T
    ).astype(f32)

    return dict(
        fcw=fcw_h, fcb=fcb_h, fc1w=fc1w_h, fc1b=fc1b_h, fc2w=fc2w_h,
        fc2b=fc2b_h, projw=projw_h, rw1w=rw1w_h, rw1b=rw1b_h,
        rw2w=rw2w_h, rw2b=rw2b_h,
    )


def _get_nc():
    if "nc" not in _CACHE:
        _CACHE["nc"] = build_nc()
    return _CACHE["nc"]


def run(inputs, trace=False, trace_kwargs=None):
    """Run the SPMD kernel; returns (full_output, BassKernelResults)."""
    x = np.asarray(inputs["x"], np.float32)
    shared = _prep_weights(
        np.asarray(inputs["fc_w"], np.float32), np.asarray(inputs["fc_b"], np.float32),
        np.asarray(inputs["fc1_w"], np.float32), np.asarray(inputs["fc1_b"], np.float32),
        np.asarray(inputs["fc2_w"], np.float32), np.asarray(inputs["fc2_b"], np.float32),
        np.asarray(inputs["rw1_w"], np.float32), np.asarray(inputs["rw1_b"], np.float32),
        np.asarray(inputs["rw2_w"], np.float32), np.asarray(inputs["rw2_b"], np.float32),
        np.asarray(inputs["proj_w"], np.float32), np.asarray(inputs["proj_b"], np.float32),
    )

    xf = x.reshape(B * T, HWTOK, C)
    in_maps = []
    for c in range(NCORES):
        sh = xf[NF * c:NF * (c + 1)]                      # [NF, 3136, 512]
        xt = sh.transpose(0, 2, 1).reshape(NF, CCB, 128, HWTOK)
        xt = np.ascontiguousarray(xt.transpose(0, 2, 1, 3)).astype(BF16_NP)
        m = dict(shared)
        m["xT"] = xt
        in_maps.append(m)

    nc = _get_nc()
    res = run_bass_kernel_spmd(
        nc, in_maps, list(range(NCORES)),
        trace=trace, **(dict(trace_kwargs=trace_kwargs) if trace_kwargs else {}),
    )

    out = np.empty((B * T, HWTOK, C), np.float32)
    for c in range(NCORES):
        out[NF * c:NF * (c + 1)] = res.results[c]["out"]
    out += np.asarray(inputs["proj_b"], np.float32)  # proj bias, host-side
    return out.reshape(B, T, H, W, C), res


def kernel(**inputs) -> np.ndarray:
    full, _ = run(inputs, trace=False)
    return full
